# revision 24
# baseline (speedup 1.0000x reference)
"""Trainium2 Bass kernel for nn_Attention_66949950210549.

Dense transformer attention block:
  qkv = x @ qkv_w.T ; per-head LN on q,k ; RoPE (positions restart at N/2) ;
  softmax(q k^T * HD^-0.5 + cross-block log(0.5) bias) @ v ; proj.

Sharding: 8 cores = 2 (batch) x 4 (head groups of 4 heads).  Each core
computes its batch's qkv for its 4 heads, attention, and a partial
projection (row-parallel over the head channels); the host sums the 4
partials per batch (the proj bias is fed to exactly one core per batch).

Per-core kernel layout strategy (all on-chip, fp16 attention core):
  - x cast to fp16, bounced through DRAM, xbar-DMA-transposed to xT (c,n)
  - qkv matmul -> natural (n, j) tiles in PSUM; LN via bn_stats/tensor_scalar,
    RoPE via host-precomputed (weight-folded) cos/sin tables; q,k bounced to
    DRAM and xbar-transposed to qT/kT (d, n) with head pairs packed into
    partitions 0-63 / 64-127 (-> 2-head row-tiled scores matmuls).
  - scoresT = kT-chunk stationary @ qT moving, per 128-key chunk; exp on ACT
    with the softmax scale and cross-block bias folded in (no max pass:
    LN bounds |score| <= 8).
  - AV: v-chunk stationary augmented with a ones column (M=65) so the
    softmax denominator accumulates in PSUM row 64 for free.
  - normalize by the reciprocal rowsum (broadcast via DRAM bounce), pack
    oT (c_in, n), project with transposed proj weights, add bias, DMA out.
"""

import math
import os
import sys

sys.path.insert(0, "/opt/trn_rl_repo")

PHASES = os.environ.get("BASS_PHASES", "ABCDEF")

import numpy as np

import concourse.bacc as bacc
import concourse.bass as bass
import concourse.tile as tile
from concourse import bass_utils, mybir

B, N, C = 2, 2048, 1024
H, HD = 16, 64
NCORES = 8
GH = 4  # head-group count (cores per batch)
NH = H // GH  # heads per core = 4
J = 3 * NH * HD  # qkv rows per core = 768
NIN = N // 2  # rope positions restart here
NT = N // 128  # 16 row tiles
CCH = C // 128  # 8 contraction chunks
LOG_COND = math.log(0.5)
EPS = 1e-5
SCALE = HD ** -0.5  # 0.125

F32 = mybir.dt.float32
F16 = mybir.dt.float16
AF = mybir.ActivationFunctionType
AX = mybir.AxisListType
ALU = mybir.AluOpType


def build_nc(with_qb: bool, with_kb: bool, shared_t: bool = False, with_pb: bool = True):
    nc = bacc.Bacc("TRN2", target_bir_lowering=False, debug=False)

    x32 = nc.dram_tensor("x32", [N, C], F32, kind="ExternalInput")
    w32 = nc.dram_tensor("w32", [J, C], F32, kind="ExternalInput")
    pw32 = nc.dram_tensor("pw32", [C, NH * HD], F32, kind="ExternalInput")
    pb = nc.dram_tensor("pb", [C], F32, kind="ExternalInput")
    nkinds_q = 6 if with_qb else 4
    nkinds_k = 6 if with_kb else 4
    nrep = 2 * NH if shared_t else NH
    tq = nc.dram_tensor("tq", [NIN, nkinds_q, nrep, 32], F16, kind="ExternalInput")
    tk = None
    if not shared_t:
        tk = nc.dram_tensor("tk", [NIN, nkinds_k, NH, 32], F16, kind="ExternalInput")
    ident = nc.dram_tensor("ident", [128, 128], F16, kind="ExternalInput")
    out_p = nc.dram_tensor("out_p", [N, C], F32, kind="ExternalOutput")

    def rng(n, ph):
        return range(n if ph in PHASES else 0)

    with tile.TileContext(nc) as tc:
        with (
            tc.tile_pool(name="persist", bufs=1) as persist,
            tc.tile_pool(name="dram", bufs=1, space="DRAM") as dram,
        ):
            # ---- persistent SBUF tensors --------------------------------
            pwT_sb = persist.tile([128, 2, C], F16)  # proj_w^T (c_in, c_out)
            pb_rep = persist.tile([128, C], F32)  # bias replicated over parts
            v_sb = persist.tile([128, NT, NH, HD + 1], F16)  # v + ones col
            qT_sb = persist.tile([128, 2, N], F16)  # head-pair packed q^T
            kT_sb = persist.tile([128, 2, N], F16)
            oT_sb = persist.tile([128, 2, N], F16)  # head-pair packed o^T

            cst = persist.tile([128, 3], F32)
            nc.vector.memset(cst[:, 0:1], EPS)
            nc.vector.memset(cst[:, 1:2], 0.0)
            nc.vector.memset(cst[:, 2:3], LOG_COND)
            nc.const_aps.aps[(F32, EPS)] = cst[:, 0:1]
            nc.const_aps.aps[(F32, 0.0)] = cst[:, 1:2]
            nc.const_aps.aps[(F32, LOG_COND)] = cst[:, 2:3]

            pb_ap = pb[:]
            pb_bcast = bass.AP(
                tensor=pb_ap.tensor,
                offset=pb_ap.offset,
                ap=[[0, 128]] + list(pb_ap.ap),
            )
            nc.gpsimd.dma_start(out=pb_rep, in_=pb_bcast)
            nc.vector.memset(v_sb[:, :, :, HD : HD + 1], 1.0)
            id_sb = persist.tile([128, 128], F16)
            nc.sync.dma_start(out=id_sb, in_=ident[:, :])

            with (
                tc.tile_pool(name="wprep", bufs=3) as wprep,
                tc.tile_pool(name="mm1", bufs=1) as mm1,
            ):
                wT_sb = mm1.tile([128, CCH, J], F16)  # qkv_w^T (c, j)
                xT_sb = mm1.tile([128, CCH, N], F16)  # x^T (c, n)
                tq_sb = mm1.tile([128, NIN // 128, nkinds_q, nrep, 32], F16)
                nc.sync.dma_start(
                    out=tq_sb, in_=tq.rearrange("(t p) k h d -> p t k h d", p=128)
                )
                tk_sb = None
                if not shared_t:
                    tk_sb = mm1.tile([128, NIN // 128, nkinds_k, NH, 32], F16)
                    nc.sync.dma_start(
                        out=tk_sb, in_=tk.rearrange("(t p) k h d -> p t k h d", p=128)
                    )

                # ---- weights: cast-on-DMA load + PE transpose -----------
                with tc.tile_pool(name="tpps", bufs=3, space="PSUM") as tpps:
                    for jt in rng(J // 128, "B"):
                        wt16 = wprep.tile([128, C], F16, tag="w16t")
                        nc.gpsimd.dma_start(
                            out=wt16, in_=w32[jt * 128 : (jt + 1) * 128, :]
                        )
                        for cg in range(2):
                            tp = tpps.tile([128, 4, 128], F16, tag="tp")
                            for k in range(4):
                                ct = cg * 4 + k
                                nc.tensor.transpose(
                                    tp[:, k, :],
                                    wt16[:, ct * 128 : (ct + 1) * 128],
                                    id_sb,
                                )
                            nc.scalar.copy(
                                out=wT_sb[
                                    :, cg * 4 : (cg + 1) * 4, jt * 128 : (jt + 1) * 128
                                ],
                                in_=tp,
                            )
                    for pt in rng(C // 128, "B"):
                        pwt16 = wprep.tile([128, NH * HD], F16, tag="pw16t")
                        nc.gpsimd.dma_start(
                            out=pwt16, in_=pw32[pt * 128 : (pt + 1) * 128, :]
                        )
                        tp = tpps.tile([128, 4, 128], F16, tag="tp")
                        for cc in range(2):
                            nc.tensor.transpose(
                                tp[:, cc, :],
                                pwt16[:, cc * 128 : (cc + 1) * 128],
                                id_sb,
                            )
                        nc.scalar.copy(
                            out=pwT_sb[:, 0:2, pt * 128 : (pt + 1) * 128],
                            in_=tp[:, 0:2, :],
                        )

                    # ---- phase C: x load/cast + PE transpose ------------
                    for i in rng(NT, "C"):
                        x16 = wprep.tile([128, C], F16, tag="x16t")
                        nc.gpsimd.dma_start(
                            out=x16, in_=x32[i * 128 : (i + 1) * 128, :]
                        )
                        for cg in range(2):
                            tp = tpps.tile([128, 4, 128], F16, tag="tp")
                            for k in range(4):
                                ct = cg * 4 + k
                                nc.tensor.transpose(
                                    tp[:, k, :],
                                    x16[:, ct * 128 : (ct + 1) * 128],
                                    id_sb,
                                )
                            nc.scalar.copy(
                                out=xT_sb[
                                    :, cg * 4 : (cg + 1) * 4, i * 128 : (i + 1) * 128
                                ],
                                in_=tp,
                            )

                # ---- phase D: qkv matmul + LN + rope ------------------------
                with (
                    tc.tile_pool(name="qkvps", bufs=3, space="PSUM") as qkvps,
                    tc.tile_pool(name="tpps2", bufs=2, space="PSUM") as tpps2,
                    tc.tile_pool(name="dwork", bufs=3) as dwork,
                ):
                    for i in rng(NT, "D"):
                        qp = qkvps.tile([128, 512], F32, tag="qp")
                        vp = qkvps.tile([128, 256], F32, tag="vp")
                        for cc in range(CCH):
                            nc.tensor.matmul(
                                qp,
                                lhsT=xT_sb[:, cc, i * 128 : (i + 1) * 128],
                                rhs=wT_sb[:, cc, 0:512],
                                start=(cc == 0),
                                stop=(cc == CCH - 1),
                            )
                        for cc in range(CCH):
                            nc.tensor.matmul(
                                vp,
                                lhsT=xT_sb[:, cc, i * 128 : (i + 1) * 128],
                                rhs=wT_sb[:, cc, 512:768],
                                start=(cc == 0),
                                stop=(cc == CCH - 1),
                            )
                        # layernorm on the 8 (q,k) head groups:
                        # grouped sums on DVE, apply on ACT (scale/bias form)
                        qk_sb = dwork.tile([128, 2 * NH, HD], F16, tag="qk")
                        sq = dwork.tile([128, 2 * NH, HD], F32, tag="sq")
                        sums = dwork.tile([128, 4, 2 * NH], F32, tag="sums")
                        qp3 = qp.rearrange("p (g d) -> p g d", g=2 * NH)
                        nc.scalar.square(out=sq, in_=qp3)
                        nc.vector.tensor_reduce(
                            out=sums[:, 0, :], in_=qp3, axis=AX.X, op=ALU.add
                        )
                        nc.vector.tensor_reduce(
                            out=sums[:, 1, :], in_=sq, axis=AX.X, op=ALU.add
                        )
                        # mu = s/64 ; var = ss/64 - mu^2 ; rstd = rsqrt(var+eps)
                        nc.vector.tensor_scalar_mul(
                            out=sums[:, 0, :], in0=sums[:, 0, :], scalar1=1.0 / HD
                        )
                        nc.vector.tensor_scalar_mul(
                            out=sums[:, 1, :], in0=sums[:, 1, :], scalar1=1.0 / HD
                        )
                        nc.vector.tensor_mul(
                            out=sums[:, 2, :], in0=sums[:, 0, :], in1=sums[:, 0, :]
                        )
                        nc.vector.tensor_sub(
                            out=sums[:, 1, :], in0=sums[:, 1, :], in1=sums[:, 2, :]
                        )
                        nc.scalar.activation(
                            out=sums[:, 1, :], in_=sums[:, 1, :], func=AF.Sqrt, bias=EPS
                        )
                        nc.vector.reciprocal(out=sums[:, 1, :], in_=sums[:, 1, :])
                        # nb = -mu * rstd  (per-partition bias for the ACT apply)
                        nc.vector.tensor_mul(
                            out=sums[:, 2, :], in0=sums[:, 0, :], in1=sums[:, 1, :]
                        )
                        nc.vector.tensor_scalar_mul(
                            out=sums[:, 2, :], in0=sums[:, 2, :], scalar1=-1.0
                        )
                        for g in range(2 * NH):
                            nc.scalar.activation(
                                out=qk_sb[:, g, :],
                                in_=qp[:, g * HD : (g + 1) * HD],
                                func=AF.Identity,
                                bias=sums[:, 2, g : g + 1],
                                scale=sums[:, 1, g : g + 1],
                            )
                        # rope (tables carry the LN weights already)
                        qkr = dwork.tile([128, 2 * NH, HD], F16, tag="qkr")
                        r = i % (NIN // 128)
                        if shared_t:
                            groups = ((tq_sb, 0, 2 * NH, with_qb),)
                        else:
                            groups = (
                                (tq_sb, 0, NH, with_qb),
                                (tk_sb, NH, NH, with_kb),
                            )
                        for tsb, base, gn, wb in groups:
                            a1 = qk_sb[:, base : base + gn, 0:32]
                            a2 = qk_sb[:, base : base + gn, 32:64]
                            o1 = qkr[:, base : base + gn, 0:32]
                            o2 = qkr[:, base : base + gn, 32:64]
                            t_full = dwork.tile(
                                [128, 2 * NH, 32], F16, tag="ropetmp", name="ropetmp"
                            )
                            t = t_full[:, 0:gn, :]
                            nc.vector.tensor_mul(out=t, in0=a1, in1=tsb[:, r, 0])
                            nc.vector.tensor_mul(out=o1, in0=a2, in1=tsb[:, r, 1])
                            nc.vector.tensor_sub(out=o1, in0=t, in1=o1)
                            nc.vector.tensor_mul(out=t, in0=a2, in1=tsb[:, r, 2])
                            nc.vector.tensor_mul(out=o2, in0=a1, in1=tsb[:, r, 3])
                            nc.vector.tensor_add(out=o2, in0=t, in1=o2)
                            if wb:
                                nc.vector.tensor_add(out=o1, in0=o1, in1=tsb[:, r, 4])
                                nc.vector.tensor_add(out=o2, in0=o2, in1=tsb[:, r, 5])
                        # qT/kT via PE transpose (head pairs packed)
                        tp = tpps2.tile([128, 4, 128], F16, tag="tpqk")
                        for hp in range(2):
                            nc.tensor.transpose(
                                tp[:, hp, :],
                                qkr[:, 2 * hp : 2 * hp + 2, :],
                                id_sb,
                            )
                            nc.tensor.transpose(
                                tp[:, 2 + hp, :],
                                qkr[:, NH + 2 * hp : NH + 2 * hp + 2, :],
                                id_sb,
                            )
                        nc.scalar.copy(
                            out=qT_sb[:, 0:2, i * 128 : (i + 1) * 128],
                            in_=tp[:, 0:2, :],
                        )
                        nc.scalar.copy(
                            out=kT_sb[:, 0:2, i * 128 : (i + 1) * 128],
                            in_=tp[:, 2:4, :],
                        )
                        # v (cast to fp16, strided into the ones-augmented slots)
                        nc.vector.tensor_copy(
                            out=v_sb[:, i, :, 0:HD],
                            in_=vp.rearrange("p (h d) -> p h d", h=NH),
                        )

            # ---- phase E: attention -------------------------------------
            with (
                tc.tile_pool(name="scps", bufs=1, space="PSUM") as scps,
                tc.tile_pool(name="avps", bufs=1, space="PSUM") as avps,
                tc.tile_pool(name="epool", bufs=2) as epool,
                tc.tile_pool(name="nwork", bufs=2) as nwork,
            ):
                for nqh in rng(2, "E"):
                    for hp in range(2):
                        e_t = [
                            epool.tile([128, NT, 1024], F16, tag=f"E{z}", name=f"E{z}")
                            for z in range(2)
                        ]
                        for kc in range(NT):
                            bias = 0.0 if ((kc < 8) == (nqh == 0)) else LOG_COND
                            for z in range(2):
                                sp = scps.tile(
                                    [128, 1024], F32, tag=f"s{z}", name=f"s{z}"
                                )
                                for nqc in range(2):
                                    nq0 = nqh * 1024 + nqc * 512
                                    nc.tensor.matmul(
                                        sp[:, nqc * 512 : (nqc + 1) * 512],
                                        lhsT=kT_sb[
                                            z * 64 : (z + 1) * 64,
                                            hp,
                                            kc * 128 : (kc + 1) * 128,
                                        ],
                                        rhs=qT_sb[
                                            z * 64 : (z + 1) * 64, hp, nq0 : nq0 + 512
                                        ],
                                        start=True,
                                        stop=True,
                                    )
                                nc.scalar.activation(
                                    out=e_t[z][:, kc, :],
                                    in_=sp,
                                    func=AF.Exp,
                                    bias=bias,
                                    scale=SCALE,
                                )
                        av_t = [
                            avps.tile([128, 1024], F32, tag=f"av{z}", name=f"av{z}")
                            for z in range(2)
                        ]
                        for kc in range(NT):
                            for z in range(2):
                                for nqc in range(2):
                                    nc.tensor.matmul(
                                        av_t[z][
                                            0 : HD + 1, nqc * 512 : (nqc + 1) * 512
                                        ],
                                        lhsT=v_sb[:, kc, 2 * hp + z, :],
                                        rhs=e_t[z][
                                            :, kc, nqc * 512 : (nqc + 1) * 512
                                        ],
                                        start=(kc == 0),
                                        stop=(kc == NT - 1),
                                    )
                        # normalize: o = av[0:64] * (1/av[64]) , pack into oT_sb
                        for z in range(2):
                            rs = nwork.tile([128, 1024], F32, tag="rs")
                            nc.vector.reciprocal(
                                out=rs[HD : HD + 1, :], in_=av_t[z][HD : HD + 1, :]
                            )
                            rs_d = dram.tile([1, 1024], F32, tag="rsd", name="rs_d")
                            nc.sync.dma_start(out=rs_d, in_=rs[HD : HD + 1, :])
                            rr = nwork.tile([64, 1024], F32, tag="rr")
                            rs_ap = rs_d[:]
                            nc.gpsimd.dma_start(
                                out=rr,
                                in_=bass.AP(
                                    tensor=rs_ap.tensor,
                                    offset=rs_ap.offset,
                                    ap=[[0, 64]] + list(rs_ap.ap[1:]),
                                ),
                            )
                            o16 = nwork.tile([64, 1024], F16, tag="o16")
                            nc.vector.tensor_mul(
                                out=o16, in0=av_t[z][0:HD, :], in1=rr
                            )
                            nc.sync.dma_start(
                                out=oT_sb[
                                    z * 64 : (z + 1) * 64,
                                    hp,
                                    nqh * 1024 : (nqh + 1) * 1024,
                                ],
                                in_=o16,
                            )

            # ---- phase F: projection ------------------------------------
            with (
                tc.tile_pool(name="prps", bufs=2, space="PSUM") as prps,
                tc.tile_pool(name="fwork", bufs=3) as fwork,
            ):
                for i in rng(NT, "F"):
                    op = prps.tile([128, C], F32, tag="op")
                    for oc in range(2):
                        for cc in range(2):
                            nc.tensor.matmul(
                                op[:, oc * 512 : (oc + 1) * 512],
                                lhsT=oT_sb[:, cc, i * 128 : (i + 1) * 128],
                                rhs=pwT_sb[:, cc, oc * 512 : (oc + 1) * 512],
                                start=(cc == 0),
                                stop=(cc == 1),
                            )
                    ot = fwork.tile([128, C], F32, tag="ot")
                    if with_pb:
                        nc.vector.tensor_add(out=ot, in0=op, in1=pb_rep)
                    else:
                        nc.scalar.copy(out=ot, in_=op)
                    nc.sync.dma_start(out=out_p[i * 128 : (i + 1) * 128, :], in_=ot)

    nc.compile()
    return nc


def _rope_tables(n_w, n_b, with_b, reps=NH):
    inv = 1.0 / (10000.0 ** (np.arange(0, HD, 2, dtype=np.float64) / HD))
    ang = np.arange(NIN, dtype=np.float64)[:, None] * inv[None, :]  # (NIN, 32)
    cos_h = np.cos(ang)
    sin_h = np.sin(ang)
    w1, w2 = n_w[:32].astype(np.float64), n_w[32:].astype(np.float64)
    b1, b2 = n_b[:32].astype(np.float64), n_b[32:].astype(np.float64)
    kinds = [w1 * cos_h, w2 * sin_h, w2 * cos_h, w1 * sin_h]
    if with_b:
        kinds += [b1 * cos_h - b2 * sin_h, b2 * cos_h + b1 * sin_h]
    t = np.stack(kinds, axis=1)  # (NIN, k, 32)
    t = np.repeat(t[:, :, None, :], reps, axis=2)  # (NIN, k, reps, 32)
    return np.ascontiguousarray(t.astype(np.float16))


_NC_CACHE = {}


def kernel(x, qkv_w, qn_w, qn_b, kn_w, kn_b, proj_w, proj_b):
    x = np.asarray(x, np.float32)
    qkv_w = np.asarray(qkv_w, np.float32)
    proj_w = np.asarray(proj_w, np.float32)
    proj_b = np.asarray(proj_b, np.float32)
    qn_w = np.asarray(qn_w, np.float32)
    qn_b = np.asarray(qn_b, np.float32)
    kn_w = np.asarray(kn_w, np.float32)
    kn_b = np.asarray(kn_b, np.float32)

    with_qb = bool(np.any(qn_b != 0))
    with_kb = bool(np.any(kn_b != 0))
    shared_t = (
        with_qb == with_kb
        and np.array_equal(qn_w, kn_w)
        and np.array_equal(qn_b, kn_b)
    )
    with_pb = bool(np.any(proj_b != 0))
    key = (with_qb, with_kb, shared_t, with_pb)
    if key not in _NC_CACHE:
        _NC_CACHE[key] = build_nc(with_qb, with_kb, shared_t, with_pb)
    nc = _NC_CACHE[key]

    tq = _rope_tables(qn_w, qn_b, with_qb, reps=2 * NH if shared_t else NH)
    tk = None if shared_t else _rope_tables(kn_w, kn_b, with_kb)
    ident = np.eye(128, dtype=np.float16)

    in_maps = []
    for core in range(NCORES):
        b, g = core // GH, core % GH
        rows = slice(g * NH * HD, (g + 1) * NH * HD)
        w_core = np.ascontiguousarray(
            np.concatenate([qkv_w[rows], qkv_w[C:][rows], qkv_w[2 * C :][rows]], 0)
        )
        im = {
            "x32": np.ascontiguousarray(x[b]),
            "w32": w_core,
            "pw32": np.ascontiguousarray(proj_w[:, rows]),
            "pb": proj_b if g == 0 else np.zeros_like(proj_b),
            "tq": tq,
            "ident": ident,
        }
        if tk is not None:
            im["tk"] = tk
        in_maps.append(im)

    res = bass_utils.run_bass_kernel_spmd(nc, in_maps, core_ids=list(range(NCORES)))
    parts = [r["out_p"] for r in res.results]
    out = np.stack(
        [np.sum(parts[b * GH : (b + 1) * GH], axis=0, dtype=np.float32) for b in range(B)]
    )
    return out.astype(np.float32)


if __name__ == "__main__":
    rng = np.random.default_rng(0)
    ins = {
        "x": rng.standard_normal((B, N, C), np.float32),
        "qkv_w": (rng.standard_normal((3 * C, C), np.float32) / math.sqrt(C)).astype(
            np.float32
        ),
        "qn_w": np.ones(HD, np.float32),
        "qn_b": np.zeros(HD, np.float32),
        "kn_w": np.ones(HD, np.float32),
        "kn_b": np.zeros(HD, np.float32),
        "proj_w": (rng.standard_normal((C, C), np.float32) / math.sqrt(C)).astype(
            np.float32
        ),
        "proj_b": np.zeros(C, np.float32),
    }
    o = kernel(**ins)
    print(o.shape, o.dtype)


# revision 28
# speedup vs baseline: 3.1232x; 3.1232x over previous
"""Trainium2 Bass kernel for nn_Attention_66949950210549.

Dense transformer attention block:
  qkv = x @ qkv_w.T ; per-head LN on q,k ; RoPE (positions restart at N/2) ;
  softmax(q k^T * HD^-0.5 + cross-block log(0.5) bias) @ v ; proj.

Sharding: 8 cores = 2 (batch) x 4 (head groups of 4 heads).  Each core
computes its batch's qkv for its 4 heads, attention, and a partial
projection (row-parallel over the head channels); the host sums the 4
partials per batch (the proj bias is fed to exactly one core per batch).

Per-core layout strategy (fp16 attention core, fp32 accumulation):
  - all loads are SWDGE cast-on-DMA (f32 DRAM -> f16 SBUF); x / qkv_w /
    proj_w are transposed on-chip with batched PE transposes (identity
    matmul), PSUM->SBUF copies routed to the otherwise-idle ACT engine.
  - qkv matmul in natural (n, j) orientation; LN via one ACT square +
    grouped DVE tensor_reduce sums, applied on ACT as Identity with
    per-partition scale=rstd, bias=-mu*rstd; RoPE via host-precomputed
    cos/sin tables with the LN weights folded in (single shared table
    when qn and kn params match).
  - q/k re-transposed to (d, n) with head PAIRS packed into partitions
    0-63 / 64-127, so the K=64 scoresT matmuls auto-row-tile into
    concurrent PE row-groups.
  - exp on ACT straight out of PSUM with softmax scale and the
    cross-block log(0.5) bias folded into the activation (no max pass:
    LN bounds |score| <= 8, exp is overflow-safe in fp32).
  - AV: v-chunk stationary augmented with a ones column (M=65) so the
    softmax denominator accumulates in PSUM row 64 for free; normalize
    by the reciprocal rowsum (partition-broadcast via a DRAM bounce),
    pack oT (c_in, n), project with transposed proj weights, DMA out.
"""

import math
import os
import sys

sys.path.insert(0, "/opt/trn_rl_repo")

PHASES = os.environ.get("BASS_PHASES", "ABCDEF")

import numpy as np

import concourse.bacc as bacc
import concourse.bass as bass
import concourse.tile as tile
from concourse import bass_utils, mybir

B, N, C = 2, 2048, 1024
H, HD = 16, 64
NCORES = 8
GH = 4  # head-group count (cores per batch)
NH = H // GH  # heads per core = 4
J = 3 * NH * HD  # qkv rows per core = 768
NIN = N // 2  # rope positions restart here
NT = N // 128  # 16 row tiles
CCH = C // 128  # 8 contraction chunks
LOG_COND = math.log(0.5)
EPS = 1e-5
SCALE = HD ** -0.5  # 0.125

F32 = mybir.dt.float32
F16 = mybir.dt.float16
AF = mybir.ActivationFunctionType
AX = mybir.AxisListType
ALU = mybir.AluOpType


def build_nc(with_qb: bool, with_kb: bool, shared_t: bool = False, with_pb: bool = True):
    nc = bacc.Bacc("TRN2", target_bir_lowering=False, debug=False)

    x32 = nc.dram_tensor("x32", [N, C], F32, kind="ExternalInput")
    w32 = nc.dram_tensor("w32", [J, C], F32, kind="ExternalInput")
    pw32 = nc.dram_tensor("pw32", [C, NH * HD], F32, kind="ExternalInput")
    pb = nc.dram_tensor("pb", [C], F32, kind="ExternalInput")
    nkinds_q = 6 if with_qb else 4
    nkinds_k = 6 if with_kb else 4
    nrep = 2 * NH if shared_t else NH
    tq = nc.dram_tensor("tq", [NIN, nkinds_q, nrep, 32], F16, kind="ExternalInput")
    tk = None
    if not shared_t:
        tk = nc.dram_tensor("tk", [NIN, nkinds_k, NH, 32], F16, kind="ExternalInput")
    ident = nc.dram_tensor("ident", [128, 128], F16, kind="ExternalInput")
    out_p = nc.dram_tensor("out_p", [N, C], F32, kind="ExternalOutput")

    def rng(n, ph):
        return range(n if ph in PHASES else 0)

    with tile.TileContext(nc) as tc:
        with (
            tc.tile_pool(name="persist", bufs=1) as persist,
            tc.tile_pool(name="dram", bufs=1, space="DRAM") as dram,
        ):
            # ---- persistent SBUF tensors --------------------------------
            pwT_sb = persist.tile([128, 2, C], F16)  # proj_w^T (c_in, c_out)
            pb_rep = persist.tile([128, C], F32)  # bias replicated over parts
            v_sb = persist.tile([128, NT, NH, HD + 1], F16)  # v + ones col
            qT_sb = persist.tile([128, 2, N], F16)  # head-pair packed q^T
            kT_sb = persist.tile([128, 2, N], F16)
            oT_sb = persist.tile([128, 2, N], F16)  # head-pair packed o^T

            cst = persist.tile([128, 3], F32)
            nc.vector.memset(cst[:, 0:1], EPS)
            nc.vector.memset(cst[:, 1:2], 0.0)
            nc.vector.memset(cst[:, 2:3], LOG_COND)
            nc.const_aps.aps[(F32, EPS)] = cst[:, 0:1]
            nc.const_aps.aps[(F32, 0.0)] = cst[:, 1:2]
            nc.const_aps.aps[(F32, LOG_COND)] = cst[:, 2:3]

            pb_ap = pb[:]
            pb_bcast = bass.AP(
                tensor=pb_ap.tensor,
                offset=pb_ap.offset,
                ap=[[0, 128]] + list(pb_ap.ap),
            )
            nc.gpsimd.dma_start(out=pb_rep, in_=pb_bcast)
            nc.vector.memset(v_sb[:, :, :, HD : HD + 1], 1.0)
            id_sb = persist.tile([128, 128], F16)
            nc.sync.dma_start(out=id_sb, in_=ident[:, :])

            with (
                tc.tile_pool(name="wprep", bufs=3) as wprep,
                tc.tile_pool(name="mm1", bufs=1) as mm1,
            ):
                wT_sb = mm1.tile([128, CCH, J], F16)  # qkv_w^T (c, j)
                xT_sb = mm1.tile([128, CCH, N], F16)  # x^T (c, n)
                tq_sb = mm1.tile([128, NIN // 128, nkinds_q, nrep, 32], F16)
                nc.sync.dma_start(
                    out=tq_sb, in_=tq.rearrange("(t p) k h d -> p t k h d", p=128)
                )
                tk_sb = None
                if not shared_t:
                    tk_sb = mm1.tile([128, NIN // 128, nkinds_k, NH, 32], F16)
                    nc.sync.dma_start(
                        out=tk_sb, in_=tk.rearrange("(t p) k h d -> p t k h d", p=128)
                    )

                # ---- weights: cast-on-DMA load + PE transpose -----------
                with tc.tile_pool(name="tpps", bufs=3, space="PSUM") as tpps:
                    for jt in rng(J // 128, "B"):
                        wt16 = wprep.tile([128, C], F16, tag="w16t")
                        nc.gpsimd.dma_start(
                            out=wt16, in_=w32[jt * 128 : (jt + 1) * 128, :]
                        )
                        for cg in range(2):
                            tp = tpps.tile([128, 4, 128], F16, tag="tp")
                            for k in range(4):
                                ct = cg * 4 + k
                                nc.tensor.transpose(
                                    tp[:, k, :],
                                    wt16[:, ct * 128 : (ct + 1) * 128],
                                    id_sb,
                                )
                            nc.scalar.copy(
                                out=wT_sb[
                                    :, cg * 4 : (cg + 1) * 4, jt * 128 : (jt + 1) * 128
                                ],
                                in_=tp,
                            )
                    for pt in rng(C // 128, "B"):
                        pwt16 = wprep.tile([128, NH * HD], F16, tag="pw16t")
                        nc.gpsimd.dma_start(
                            out=pwt16, in_=pw32[pt * 128 : (pt + 1) * 128, :]
                        )
                        tp = tpps.tile([128, 4, 128], F16, tag="tp")
                        for cc in range(2):
                            nc.tensor.transpose(
                                tp[:, cc, :],
                                pwt16[:, cc * 128 : (cc + 1) * 128],
                                id_sb,
                            )
                        nc.scalar.copy(
                            out=pwT_sb[:, 0:2, pt * 128 : (pt + 1) * 128],
                            in_=tp[:, 0:2, :],
                        )

                    # ---- phase C: x load/cast + PE transpose ------------
                    for i in rng(NT, "C"):
                        x16 = wprep.tile([128, C], F16, tag="x16t")
                        nc.gpsimd.dma_start(
                            out=x16, in_=x32[i * 128 : (i + 1) * 128, :]
                        )
                        for cg in range(2):
                            tp = tpps.tile([128, 4, 128], F16, tag="tp")
                            for k in range(4):
                                ct = cg * 4 + k
                                nc.tensor.transpose(
                                    tp[:, k, :],
                                    x16[:, ct * 128 : (ct + 1) * 128],
                                    id_sb,
                                )
                            nc.scalar.copy(
                                out=xT_sb[
                                    :, cg * 4 : (cg + 1) * 4, i * 128 : (i + 1) * 128
                                ],
                                in_=tp,
                            )

                # ---- phase D: qkv matmul + LN + rope ------------------------
                with (
                    tc.tile_pool(name="qkvps", bufs=3, space="PSUM") as qkvps,
                    tc.tile_pool(name="tpps2", bufs=2, space="PSUM") as tpps2,
                    tc.tile_pool(name="dwork", bufs=3) as dwork,
                ):
                    for i in rng(NT, "D"):
                        qp = qkvps.tile([128, 512], F32, tag="qp")
                        vp = qkvps.tile([128, 256], F32, tag="vp")
                        for cc in range(CCH):
                            nc.tensor.matmul(
                                qp,
                                lhsT=xT_sb[:, cc, i * 128 : (i + 1) * 128],
                                rhs=wT_sb[:, cc, 0:512],
                                start=(cc == 0),
                                stop=(cc == CCH - 1),
                            )
                            nc.tensor.matmul(
                                vp,
                                lhsT=xT_sb[:, cc, i * 128 : (i + 1) * 128],
                                rhs=wT_sb[:, cc, 512:768],
                                start=(cc == 0),
                                stop=(cc == CCH - 1),
                            )
                        # layernorm on the 8 (q,k) head groups:
                        # grouped sums on DVE, apply on ACT (scale/bias form)
                        qk_sb = dwork.tile([128, 2 * NH, HD], F16, tag="qk")
                        sq = dwork.tile([128, 2 * NH, HD], F32, tag="sq")
                        sums = dwork.tile([128, 4, 2 * NH], F32, tag="sums")
                        qp3 = qp.rearrange("p (g d) -> p g d", g=2 * NH)
                        nc.scalar.square(out=sq, in_=qp3)
                        nc.vector.tensor_reduce(
                            out=sums[:, 0, :], in_=qp3, axis=AX.X, op=ALU.add
                        )
                        nc.vector.tensor_reduce(
                            out=sums[:, 1, :], in_=sq, axis=AX.X, op=ALU.add
                        )
                        # mu = s/64 ; var = ss/64 - mu^2 ; rstd = rsqrt(var+eps)
                        nc.vector.tensor_scalar_mul(
                            out=sums[:, 0, :], in0=sums[:, 0, :], scalar1=1.0 / HD
                        )
                        nc.vector.tensor_scalar_mul(
                            out=sums[:, 1, :], in0=sums[:, 1, :], scalar1=1.0 / HD
                        )
                        nc.vector.tensor_mul(
                            out=sums[:, 2, :], in0=sums[:, 0, :], in1=sums[:, 0, :]
                        )
                        nc.vector.tensor_sub(
                            out=sums[:, 1, :], in0=sums[:, 1, :], in1=sums[:, 2, :]
                        )
                        nc.scalar.activation(
                            out=sums[:, 1, :], in_=sums[:, 1, :], func=AF.Sqrt, bias=EPS
                        )
                        nc.vector.reciprocal(out=sums[:, 1, :], in_=sums[:, 1, :])
                        # nb = -mu * rstd  (per-partition bias for the ACT apply)
                        nc.vector.tensor_mul(
                            out=sums[:, 2, :], in0=sums[:, 0, :], in1=sums[:, 1, :]
                        )
                        nc.vector.tensor_scalar_mul(
                            out=sums[:, 2, :], in0=sums[:, 2, :], scalar1=-1.0
                        )
                        for g in range(2 * NH):
                            nc.scalar.activation(
                                out=qk_sb[:, g, :],
                                in_=qp[:, g * HD : (g + 1) * HD],
                                func=AF.Identity,
                                bias=sums[:, 2, g : g + 1],
                                scale=sums[:, 1, g : g + 1],
                            )
                        # rope (tables carry the LN weights already)
                        qkr = dwork.tile([128, 2 * NH, HD], F16, tag="qkr")
                        r = i % (NIN // 128)
                        if shared_t:
                            groups = ((tq_sb, 0, 2 * NH, with_qb),)
                        else:
                            groups = (
                                (tq_sb, 0, NH, with_qb),
                                (tk_sb, NH, NH, with_kb),
                            )
                        for tsb, base, gn, wb in groups:
                            a1 = qk_sb[:, base : base + gn, 0:32]
                            a2 = qk_sb[:, base : base + gn, 32:64]
                            o1 = qkr[:, base : base + gn, 0:32]
                            o2 = qkr[:, base : base + gn, 32:64]
                            t_full = dwork.tile(
                                [128, 2 * NH, 32], F16, tag="ropetmp", name="ropetmp"
                            )
                            t = t_full[:, 0:gn, :]
                            nc.vector.tensor_mul(out=t, in0=a1, in1=tsb[:, r, 0])
                            nc.vector.tensor_mul(out=o1, in0=a2, in1=tsb[:, r, 1])
                            nc.vector.tensor_sub(out=o1, in0=t, in1=o1)
                            nc.vector.tensor_mul(out=t, in0=a2, in1=tsb[:, r, 2])
                            nc.vector.tensor_mul(out=o2, in0=a1, in1=tsb[:, r, 3])
                            nc.vector.tensor_add(out=o2, in0=t, in1=o2)
                            if wb:
                                nc.vector.tensor_add(out=o1, in0=o1, in1=tsb[:, r, 4])
                                nc.vector.tensor_add(out=o2, in0=o2, in1=tsb[:, r, 5])
                        # qT/kT via PE transpose (head pairs packed)
                        tp = tpps2.tile([128, 4, 128], F16, tag="tpqk")
                        for hp in range(2):
                            nc.tensor.transpose(
                                tp[:, hp, :],
                                qkr[:, 2 * hp : 2 * hp + 2, :],
                                id_sb,
                            )
                            nc.tensor.transpose(
                                tp[:, 2 + hp, :],
                                qkr[:, NH + 2 * hp : NH + 2 * hp + 2, :],
                                id_sb,
                            )
                        nc.scalar.copy(
                            out=qT_sb[:, 0:2, i * 128 : (i + 1) * 128],
                            in_=tp[:, 0:2, :],
                        )
                        nc.scalar.copy(
                            out=kT_sb[:, 0:2, i * 128 : (i + 1) * 128],
                            in_=tp[:, 2:4, :],
                        )
                        # v (cast to fp16, strided into the ones-augmented slots)
                        nc.vector.tensor_copy(
                            out=v_sb[:, i, :, 0:HD],
                            in_=vp.rearrange("p (h d) -> p h d", h=NH),
                        )

            # ---- phase E: attention -------------------------------------
            with (
                tc.tile_pool(name="scps", bufs=1, space="PSUM") as scps,
                tc.tile_pool(name="avps", bufs=1, space="PSUM") as avps,
                tc.tile_pool(name="epool", bufs=2) as epool,
                tc.tile_pool(name="nwork", bufs=2) as nwork,
            ):
                for nqh in rng(2, "E"):
                    for hp in range(2):
                        e_t = [
                            epool.tile([128, NT, 1024], F16, tag=f"E{z}", name=f"E{z}")
                            for z in range(2)
                        ]
                        for kc in range(NT):
                            bias = 0.0 if ((kc < 8) == (nqh == 0)) else LOG_COND
                            for z in range(2):
                                sp = scps.tile(
                                    [128, 1024], F32, tag=f"s{z}", name=f"s{z}"
                                )
                                for nqc in range(2):
                                    nq0 = nqh * 1024 + nqc * 512
                                    nc.tensor.matmul(
                                        sp[:, nqc * 512 : (nqc + 1) * 512],
                                        lhsT=kT_sb[
                                            z * 64 : (z + 1) * 64,
                                            hp,
                                            kc * 128 : (kc + 1) * 128,
                                        ],
                                        rhs=qT_sb[
                                            z * 64 : (z + 1) * 64, hp, nq0 : nq0 + 512
                                        ],
                                        start=True,
                                        stop=True,
                                    )
                                nc.scalar.activation(
                                    out=e_t[z][:, kc, :],
                                    in_=sp,
                                    func=AF.Exp,
                                    bias=bias,
                                    scale=SCALE,
                                )
                        av_t = [
                            avps.tile([128, 1024], F32, tag=f"av{z}", name=f"av{z}")
                            for z in range(2)
                        ]
                        for kc in range(NT):
                            for z in range(2):
                                for nqc in range(2):
                                    nc.tensor.matmul(
                                        av_t[z][
                                            0 : HD + 1, nqc * 512 : (nqc + 1) * 512
                                        ],
                                        lhsT=v_sb[:, kc, 2 * hp + z, :],
                                        rhs=e_t[z][
                                            :, kc, nqc * 512 : (nqc + 1) * 512
                                        ],
                                        start=(kc == 0),
                                        stop=(kc == NT - 1),
                                    )
                        # normalize: o = av[0:64] * (1/av[64]) , pack into oT_sb
                        for z in range(2):
                            rs = nwork.tile([128, 1024], F32, tag="rs")
                            nc.vector.reciprocal(
                                out=rs[HD : HD + 1, :], in_=av_t[z][HD : HD + 1, :]
                            )
                            rs_d = dram.tile([1, 1024], F32, tag="rsd", name="rs_d")
                            nc.sync.dma_start(out=rs_d, in_=rs[HD : HD + 1, :])
                            rr = nwork.tile([64, 1024], F32, tag="rr")
                            rs_ap = rs_d[:]
                            nc.gpsimd.dma_start(
                                out=rr,
                                in_=bass.AP(
                                    tensor=rs_ap.tensor,
                                    offset=rs_ap.offset,
                                    ap=[[0, 64]] + list(rs_ap.ap[1:]),
                                ),
                            )
                            o16 = nwork.tile([64, 1024], F16, tag="o16")
                            nc.vector.tensor_mul(
                                out=o16, in0=av_t[z][0:HD, :], in1=rr
                            )
                            nc.sync.dma_start(
                                out=oT_sb[
                                    z * 64 : (z + 1) * 64,
                                    hp,
                                    nqh * 1024 : (nqh + 1) * 1024,
                                ],
                                in_=o16,
                            )

            # ---- phase F: projection ------------------------------------
            with (
                tc.tile_pool(name="prps", bufs=2, space="PSUM") as prps,
                tc.tile_pool(name="fwork", bufs=3) as fwork,
            ):
                for i in rng(NT, "F"):
                    op = prps.tile([128, C], F32, tag="op")
                    for cc in range(2):
                        for oc in range(2):
                            nc.tensor.matmul(
                                op[:, oc * 512 : (oc + 1) * 512],
                                lhsT=oT_sb[:, cc, i * 128 : (i + 1) * 128],
                                rhs=pwT_sb[:, cc, oc * 512 : (oc + 1) * 512],
                                start=(cc == 0),
                                stop=(cc == 1),
                            )
                    ot = fwork.tile([128, C], F32, tag="ot")
                    if with_pb:
                        nc.vector.tensor_add(out=ot, in0=op, in1=pb_rep)
                    else:
                        nc.scalar.copy(out=ot, in_=op)
                    nc.sync.dma_start(out=out_p[i * 128 : (i + 1) * 128, :], in_=ot)

    nc.compile()
    return nc


def _rope_tables(n_w, n_b, with_b, reps=NH):
    inv = 1.0 / (10000.0 ** (np.arange(0, HD, 2, dtype=np.float64) / HD))
    ang = np.arange(NIN, dtype=np.float64)[:, None] * inv[None, :]  # (NIN, 32)
    cos_h = np.cos(ang)
    sin_h = np.sin(ang)
    w1, w2 = n_w[:32].astype(np.float64), n_w[32:].astype(np.float64)
    b1, b2 = n_b[:32].astype(np.float64), n_b[32:].astype(np.float64)
    kinds = [w1 * cos_h, w2 * sin_h, w2 * cos_h, w1 * sin_h]
    if with_b:
        kinds += [b1 * cos_h - b2 * sin_h, b2 * cos_h + b1 * sin_h]
    t = np.stack(kinds, axis=1)  # (NIN, k, 32)
    t = np.repeat(t[:, :, None, :], reps, axis=2)  # (NIN, k, reps, 32)
    return np.ascontiguousarray(t.astype(np.float16))


_NC_CACHE = {}


def kernel(x, qkv_w, qn_w, qn_b, kn_w, kn_b, proj_w, proj_b):
    x = np.asarray(x, np.float32)
    qkv_w = np.asarray(qkv_w, np.float32)
    proj_w = np.asarray(proj_w, np.float32)
    proj_b = np.asarray(proj_b, np.float32)
    qn_w = np.asarray(qn_w, np.float32)
    qn_b = np.asarray(qn_b, np.float32)
    kn_w = np.asarray(kn_w, np.float32)
    kn_b = np.asarray(kn_b, np.float32)

    with_qb = bool(np.any(qn_b != 0))
    with_kb = bool(np.any(kn_b != 0))
    shared_t = (
        with_qb == with_kb
        and np.array_equal(qn_w, kn_w)
        and np.array_equal(qn_b, kn_b)
    )
    with_pb = bool(np.any(proj_b != 0))
    key = (with_qb, with_kb, shared_t, with_pb)
    if key not in _NC_CACHE:
        _NC_CACHE[key] = build_nc(with_qb, with_kb, shared_t, with_pb)
    nc = _NC_CACHE[key]

    tq = _rope_tables(qn_w, qn_b, with_qb, reps=2 * NH if shared_t else NH)
    tk = None if shared_t else _rope_tables(kn_w, kn_b, with_kb)
    ident = np.eye(128, dtype=np.float16)

    in_maps = []
    for core in range(NCORES):
        b, g = core // GH, core % GH
        rows = slice(g * NH * HD, (g + 1) * NH * HD)
        w_core = np.ascontiguousarray(
            np.concatenate([qkv_w[rows], qkv_w[C:][rows], qkv_w[2 * C :][rows]], 0)
        )
        im = {
            "x32": np.ascontiguousarray(x[b]),
            "w32": w_core,
            "pw32": np.ascontiguousarray(proj_w[:, rows]),
            "pb": proj_b if g == 0 else np.zeros_like(proj_b),
            "tq": tq,
            "ident": ident,
        }
        if tk is not None:
            im["tk"] = tk
        in_maps.append(im)

    res = bass_utils.run_bass_kernel_spmd(nc, in_maps, core_ids=list(range(NCORES)))
    parts = [r["out_p"] for r in res.results]
    out = np.stack(
        [np.sum(parts[b * GH : (b + 1) * GH], axis=0, dtype=np.float32) for b in range(B)]
    )
    return out.astype(np.float32)


if __name__ == "__main__":
    rng = np.random.default_rng(0)
    ins = {
        "x": rng.standard_normal((B, N, C), np.float32),
        "qkv_w": (rng.standard_normal((3 * C, C), np.float32) / math.sqrt(C)).astype(
            np.float32
        ),
        "qn_w": np.ones(HD, np.float32),
        "qn_b": np.zeros(HD, np.float32),
        "kn_w": np.ones(HD, np.float32),
        "kn_b": np.zeros(HD, np.float32),
        "proj_w": (rng.standard_normal((C, C), np.float32) / math.sqrt(C)).astype(
            np.float32
        ),
        "proj_b": np.zeros(C, np.float32),
    }
    o = kernel(**ins)
    print(o.shape, o.dtype)


# revision 29
# speedup vs baseline: 3.1496x; 1.0085x over previous
"""Trainium2 Bass kernel for nn_Attention_66949950210549.

Dense transformer attention block:
  qkv = x @ qkv_w.T ; per-head LN on q,k ; RoPE (positions restart at N/2) ;
  softmax(q k^T * HD^-0.5 + cross-block log(0.5) bias) @ v ; proj.

Sharding: 8 cores = 2 (batch) x 4 (head groups of 4 heads).  Each core
computes its batch's qkv for its 4 heads, attention, and a partial
projection (row-parallel over the head channels); the host sums the 4
partials per batch (the proj bias is fed to exactly one core per batch).

Per-core layout strategy (fp16 attention core, fp32 accumulation):
  - all loads are SWDGE cast-on-DMA (f32 DRAM -> f16 SBUF); x / qkv_w /
    proj_w are transposed on-chip with batched PE transposes (identity
    matmul), PSUM->SBUF copies routed to the otherwise-idle ACT engine.
  - qkv matmul in natural (n, j) orientation; LN via one ACT square +
    grouped DVE tensor_reduce sums, applied on ACT as Identity with
    per-partition scale=rstd, bias=-mu*rstd; RoPE via host-precomputed
    cos/sin tables with the LN weights folded in (single shared table
    when qn and kn params match).
  - q/k re-transposed to (d, n) with head PAIRS packed into partitions
    0-63 / 64-127, so the K=64 scoresT matmuls auto-row-tile into
    concurrent PE row-groups.
  - exp on ACT straight out of PSUM with softmax scale and the
    cross-block log(0.5) bias folded into the activation (no max pass:
    LN bounds |score| <= 8, exp is overflow-safe in fp32).
  - AV: v-chunk stationary augmented with a ones column (M=65) so the
    softmax denominator accumulates in PSUM row 64 for free; normalize
    by the reciprocal rowsum (partition-broadcast via a DRAM bounce),
    pack oT (c_in, n), project with transposed proj weights, DMA out.
"""

import math
import os
import sys

sys.path.insert(0, "/opt/trn_rl_repo")

PHASES = os.environ.get("BASS_PHASES", "ABCDEF")

import numpy as np

import concourse.bacc as bacc
import concourse.bass as bass
import concourse.tile as tile
from concourse import bass_utils, mybir

B, N, C = 2, 2048, 1024
H, HD = 16, 64
NCORES = 8
GH = 4  # head-group count (cores per batch)
NH = H // GH  # heads per core = 4
J = 3 * NH * HD  # qkv rows per core = 768
NIN = N // 2  # rope positions restart here
NT = N // 128  # 16 row tiles
CCH = C // 128  # 8 contraction chunks
LOG_COND = math.log(0.5)
EPS = 1e-5
SCALE = HD ** -0.5  # 0.125

F32 = mybir.dt.float32
F16 = mybir.dt.float16
AF = mybir.ActivationFunctionType
AX = mybir.AxisListType
ALU = mybir.AluOpType


def build_nc(with_qb: bool, with_kb: bool, shared_t: bool = False, with_pb: bool = True):
    nc = bacc.Bacc("TRN2", target_bir_lowering=False, debug=False)

    x32 = nc.dram_tensor("x32", [N, C], F32, kind="ExternalInput")
    w32 = nc.dram_tensor("w32", [J, C], F32, kind="ExternalInput")
    pw32 = nc.dram_tensor("pw32", [C, NH * HD], F32, kind="ExternalInput")
    pb = nc.dram_tensor("pb", [C], F32, kind="ExternalInput")
    nkinds_q = 6 if with_qb else 4
    nkinds_k = 6 if with_kb else 4
    nrep = 2 * NH if shared_t else NH
    tq = nc.dram_tensor("tq", [NIN, nkinds_q, nrep, 32], F16, kind="ExternalInput")
    tk = None
    if not shared_t:
        tk = nc.dram_tensor("tk", [NIN, nkinds_k, NH, 32], F16, kind="ExternalInput")
    ident = nc.dram_tensor("ident", [128, 128], F16, kind="ExternalInput")
    out_p = nc.dram_tensor("out_p", [N, C], F32, kind="ExternalOutput")

    def rng(n, ph):
        return range(n if ph in PHASES else 0)

    with tile.TileContext(nc) as tc:
        with (
            tc.tile_pool(name="persist", bufs=1) as persist,
            tc.tile_pool(name="dram", bufs=1, space="DRAM") as dram,
        ):
            # ---- persistent SBUF tensors --------------------------------
            pwT_sb = persist.tile([128, 2, C], F16)  # proj_w^T (c_in, c_out)
            pb_rep = persist.tile([128, C], F32)  # bias replicated over parts
            v_sb = persist.tile([128, NT, NH, HD + 1], F16)  # v + ones col
            qT_sb = persist.tile([128, 2, N], F16)  # head-pair packed q^T
            kT_sb = persist.tile([128, 2, N], F16)
            oT_sb = persist.tile([128, 2, N], F16)  # head-pair packed o^T

            cst = persist.tile([128, 3], F32)
            nc.vector.memset(cst[:, 0:1], EPS)
            nc.vector.memset(cst[:, 1:2], 0.0)
            nc.vector.memset(cst[:, 2:3], LOG_COND)
            nc.const_aps.aps[(F32, EPS)] = cst[:, 0:1]
            nc.const_aps.aps[(F32, 0.0)] = cst[:, 1:2]
            nc.const_aps.aps[(F32, LOG_COND)] = cst[:, 2:3]

            pb_ap = pb[:]
            pb_bcast = bass.AP(
                tensor=pb_ap.tensor,
                offset=pb_ap.offset,
                ap=[[0, 128]] + list(pb_ap.ap),
            )
            nc.gpsimd.dma_start(out=pb_rep, in_=pb_bcast)
            nc.vector.memset(v_sb[:, :, :, HD : HD + 1], 1.0)
            id_sb = persist.tile([128, 128], F16)
            nc.sync.dma_start(out=id_sb, in_=ident[:, :])

            with (
                tc.tile_pool(name="wprep", bufs=4) as wprep,
                tc.tile_pool(name="mm1", bufs=1) as mm1,
            ):
                wT_sb = mm1.tile([128, CCH, J], F16)  # qkv_w^T (c, j)
                xT_sb = mm1.tile([128, CCH, N], F16)  # x^T (c, n)
                tq_sb = mm1.tile([128, NIN // 128, nkinds_q, nrep, 32], F16)
                nc.sync.dma_start(
                    out=tq_sb, in_=tq.rearrange("(t p) k h d -> p t k h d", p=128)
                )
                tk_sb = None
                if not shared_t:
                    tk_sb = mm1.tile([128, NIN // 128, nkinds_k, NH, 32], F16)
                    nc.sync.dma_start(
                        out=tk_sb, in_=tk.rearrange("(t p) k h d -> p t k h d", p=128)
                    )

                # ---- weights: cast-on-DMA load + PE transpose -----------
                with tc.tile_pool(name="tpps", bufs=3, space="PSUM") as tpps:
                    for jt in rng(J // 128, "B"):
                        wt16 = wprep.tile([128, C], F16, tag="w16t")
                        nc.gpsimd.dma_start(
                            out=wt16, in_=w32[jt * 128 : (jt + 1) * 128, :]
                        )
                        for cg in range(2):
                            tp = tpps.tile([128, 4, 128], F16, tag="tp")
                            for k in range(4):
                                ct = cg * 4 + k
                                nc.tensor.transpose(
                                    tp[:, k, :],
                                    wt16[:, ct * 128 : (ct + 1) * 128],
                                    id_sb,
                                )
                            nc.scalar.copy(
                                out=wT_sb[
                                    :, cg * 4 : (cg + 1) * 4, jt * 128 : (jt + 1) * 128
                                ],
                                in_=tp,
                            )
                    for pt in rng(C // 128, "B"):
                        pwt16 = wprep.tile([128, NH * HD], F16, tag="pw16t")
                        nc.gpsimd.dma_start(
                            out=pwt16, in_=pw32[pt * 128 : (pt + 1) * 128, :]
                        )
                        tp = tpps.tile([128, 4, 128], F16, tag="tp")
                        for cc in range(2):
                            nc.tensor.transpose(
                                tp[:, cc, :],
                                pwt16[:, cc * 128 : (cc + 1) * 128],
                                id_sb,
                            )
                        nc.scalar.copy(
                            out=pwT_sb[:, 0:2, pt * 128 : (pt + 1) * 128],
                            in_=tp[:, 0:2, :],
                        )

                    # ---- phase C: x load/cast + PE transpose ------------
                    for i in rng(NT, "C"):
                        x16 = wprep.tile([128, C], F16, tag="x16t")
                        nc.gpsimd.dma_start(
                            out=x16, in_=x32[i * 128 : (i + 1) * 128, :]
                        )
                        for cg in range(2):
                            tp = tpps.tile([128, 4, 128], F16, tag="tp")
                            for k in range(4):
                                ct = cg * 4 + k
                                nc.tensor.transpose(
                                    tp[:, k, :],
                                    x16[:, ct * 128 : (ct + 1) * 128],
                                    id_sb,
                                )
                            nc.scalar.copy(
                                out=xT_sb[
                                    :, cg * 4 : (cg + 1) * 4, i * 128 : (i + 1) * 128
                                ],
                                in_=tp,
                            )

                # ---- phase D: qkv matmul + LN + rope ------------------------
                with (
                    tc.tile_pool(name="qkvps", bufs=3, space="PSUM") as qkvps,
                    tc.tile_pool(name="tpps2", bufs=2, space="PSUM") as tpps2,
                    tc.tile_pool(name="dwork", bufs=4) as dwork,
                ):
                    for i in rng(NT, "D"):
                        qp = qkvps.tile([128, 512], F32, tag="qp")
                        vp = qkvps.tile([128, 256], F32, tag="vp")
                        for cc in range(CCH):
                            nc.tensor.matmul(
                                qp,
                                lhsT=xT_sb[:, cc, i * 128 : (i + 1) * 128],
                                rhs=wT_sb[:, cc, 0:512],
                                start=(cc == 0),
                                stop=(cc == CCH - 1),
                            )
                            nc.tensor.matmul(
                                vp,
                                lhsT=xT_sb[:, cc, i * 128 : (i + 1) * 128],
                                rhs=wT_sb[:, cc, 512:768],
                                start=(cc == 0),
                                stop=(cc == CCH - 1),
                            )
                        # layernorm on the 8 (q,k) head groups:
                        # grouped sums on DVE, apply on ACT (scale/bias form)
                        qk_sb = dwork.tile([128, 2 * NH, HD], F16, tag="qk")
                        sq = dwork.tile([128, 2 * NH, HD], F32, tag="sq")
                        sums = dwork.tile([128, 4, 2 * NH], F32, tag="sums")
                        qp3 = qp.rearrange("p (g d) -> p g d", g=2 * NH)
                        nc.scalar.square(out=sq, in_=qp3)
                        nc.vector.tensor_reduce(
                            out=sums[:, 0, :], in_=qp3, axis=AX.X, op=ALU.add
                        )
                        nc.vector.tensor_reduce(
                            out=sums[:, 1, :], in_=sq, axis=AX.X, op=ALU.add
                        )
                        # mu = s/64 ; var = ss/64 - mu^2 ; rstd = rsqrt(var+eps)
                        nc.vector.tensor_scalar_mul(
                            out=sums[:, 0, :], in0=sums[:, 0, :], scalar1=1.0 / HD
                        )
                        nc.vector.tensor_scalar_mul(
                            out=sums[:, 1, :], in0=sums[:, 1, :], scalar1=1.0 / HD
                        )
                        nc.vector.tensor_mul(
                            out=sums[:, 2, :], in0=sums[:, 0, :], in1=sums[:, 0, :]
                        )
                        nc.vector.tensor_sub(
                            out=sums[:, 1, :], in0=sums[:, 1, :], in1=sums[:, 2, :]
                        )
                        nc.scalar.activation(
                            out=sums[:, 1, :], in_=sums[:, 1, :], func=AF.Sqrt, bias=EPS
                        )
                        nc.vector.reciprocal(out=sums[:, 1, :], in_=sums[:, 1, :])
                        # nb = -mu * rstd  (per-partition bias for the ACT apply)
                        nc.vector.tensor_mul(
                            out=sums[:, 2, :], in0=sums[:, 0, :], in1=sums[:, 1, :]
                        )
                        nc.vector.tensor_scalar_mul(
                            out=sums[:, 2, :], in0=sums[:, 2, :], scalar1=-1.0
                        )
                        for g in range(2 * NH):
                            nc.scalar.activation(
                                out=qk_sb[:, g, :],
                                in_=qp[:, g * HD : (g + 1) * HD],
                                func=AF.Identity,
                                bias=sums[:, 2, g : g + 1],
                                scale=sums[:, 1, g : g + 1],
                            )
                        # rope (tables carry the LN weights already)
                        qkr = dwork.tile([128, 2 * NH, HD], F16, tag="qkr")
                        r = i % (NIN // 128)
                        if shared_t:
                            groups = ((tq_sb, 0, 2 * NH, with_qb),)
                        else:
                            groups = (
                                (tq_sb, 0, NH, with_qb),
                                (tk_sb, NH, NH, with_kb),
                            )
                        for tsb, base, gn, wb in groups:
                            a1 = qk_sb[:, base : base + gn, 0:32]
                            a2 = qk_sb[:, base : base + gn, 32:64]
                            o1 = qkr[:, base : base + gn, 0:32]
                            o2 = qkr[:, base : base + gn, 32:64]
                            t_full = dwork.tile(
                                [128, 2 * NH, 32], F16, tag="ropetmp", name="ropetmp"
                            )
                            t = t_full[:, 0:gn, :]
                            nc.vector.tensor_mul(out=t, in0=a1, in1=tsb[:, r, 0])
                            nc.vector.tensor_mul(out=o1, in0=a2, in1=tsb[:, r, 1])
                            nc.vector.tensor_sub(out=o1, in0=t, in1=o1)
                            nc.vector.tensor_mul(out=t, in0=a2, in1=tsb[:, r, 2])
                            nc.vector.tensor_mul(out=o2, in0=a1, in1=tsb[:, r, 3])
                            nc.vector.tensor_add(out=o2, in0=t, in1=o2)
                            if wb:
                                nc.vector.tensor_add(out=o1, in0=o1, in1=tsb[:, r, 4])
                                nc.vector.tensor_add(out=o2, in0=o2, in1=tsb[:, r, 5])
                        # qT/kT via PE transpose (head pairs packed)
                        tp = tpps2.tile([128, 4, 128], F16, tag="tpqk")
                        for hp in range(2):
                            nc.tensor.transpose(
                                tp[:, hp, :],
                                qkr[:, 2 * hp : 2 * hp + 2, :],
                                id_sb,
                            )
                            nc.tensor.transpose(
                                tp[:, 2 + hp, :],
                                qkr[:, NH + 2 * hp : NH + 2 * hp + 2, :],
                                id_sb,
                            )
                        nc.scalar.copy(
                            out=qT_sb[:, 0:2, i * 128 : (i + 1) * 128],
                            in_=tp[:, 0:2, :],
                        )
                        nc.scalar.copy(
                            out=kT_sb[:, 0:2, i * 128 : (i + 1) * 128],
                            in_=tp[:, 2:4, :],
                        )
                        # v (cast to fp16, strided into the ones-augmented slots)
                        nc.vector.tensor_copy(
                            out=v_sb[:, i, :, 0:HD],
                            in_=vp.rearrange("p (h d) -> p h d", h=NH),
                        )

            # ---- phase E: attention -------------------------------------
            with (
                tc.tile_pool(name="scps", bufs=1, space="PSUM") as scps,
                tc.tile_pool(name="avps", bufs=1, space="PSUM") as avps,
                tc.tile_pool(name="epool", bufs=2) as epool,
                tc.tile_pool(name="nwork", bufs=2) as nwork,
            ):
                for nqh in rng(2, "E"):
                    for hp in range(2):
                        e_t = [
                            epool.tile([128, NT, 1024], F16, tag=f"E{z}", name=f"E{z}")
                            for z in range(2)
                        ]
                        for kc in range(NT):
                            bias = 0.0 if ((kc < 8) == (nqh == 0)) else LOG_COND
                            for z in range(2):
                                sp = scps.tile(
                                    [128, 1024], F32, tag=f"s{z}", name=f"s{z}"
                                )
                                for nqc in range(2):
                                    nq0 = nqh * 1024 + nqc * 512
                                    nc.tensor.matmul(
                                        sp[:, nqc * 512 : (nqc + 1) * 512],
                                        lhsT=kT_sb[
                                            z * 64 : (z + 1) * 64,
                                            hp,
                                            kc * 128 : (kc + 1) * 128,
                                        ],
                                        rhs=qT_sb[
                                            z * 64 : (z + 1) * 64, hp, nq0 : nq0 + 512
                                        ],
                                        start=True,
                                        stop=True,
                                    )
                                nc.scalar.activation(
                                    out=e_t[z][:, kc, :],
                                    in_=sp,
                                    func=AF.Exp,
                                    bias=bias,
                                    scale=SCALE,
                                )
                        av_t = [
                            avps.tile([128, 1024], F32, tag=f"av{z}", name=f"av{z}")
                            for z in range(2)
                        ]
                        for kc in range(NT):
                            for z in range(2):
                                for nqc in range(2):
                                    nc.tensor.matmul(
                                        av_t[z][
                                            0 : HD + 1, nqc * 512 : (nqc + 1) * 512
                                        ],
                                        lhsT=v_sb[:, kc, 2 * hp + z, :],
                                        rhs=e_t[z][
                                            :, kc, nqc * 512 : (nqc + 1) * 512
                                        ],
                                        start=(kc == 0),
                                        stop=(kc == NT - 1),
                                    )
                        # normalize: o = av[0:64] * (1/av[64]) , pack into oT_sb
                        for z in range(2):
                            rs = nwork.tile([128, 1024], F32, tag="rs")
                            nc.vector.reciprocal(
                                out=rs[HD : HD + 1, :], in_=av_t[z][HD : HD + 1, :]
                            )
                            rs_d = dram.tile([1, 1024], F32, tag="rsd", name="rs_d")
                            nc.sync.dma_start(out=rs_d, in_=rs[HD : HD + 1, :])
                            rr = nwork.tile([64, 1024], F32, tag="rr")
                            rs_ap = rs_d[:]
                            nc.gpsimd.dma_start(
                                out=rr,
                                in_=bass.AP(
                                    tensor=rs_ap.tensor,
                                    offset=rs_ap.offset,
                                    ap=[[0, 64]] + list(rs_ap.ap[1:]),
                                ),
                            )
                            o16 = nwork.tile([64, 1024], F16, tag="o16")
                            nc.vector.tensor_mul(
                                out=o16, in0=av_t[z][0:HD, :], in1=rr
                            )
                            nc.sync.dma_start(
                                out=oT_sb[
                                    z * 64 : (z + 1) * 64,
                                    hp,
                                    nqh * 1024 : (nqh + 1) * 1024,
                                ],
                                in_=o16,
                            )

            # ---- phase F: projection ------------------------------------
            with (
                tc.tile_pool(name="prps", bufs=2, space="PSUM") as prps,
                tc.tile_pool(name="fwork", bufs=3) as fwork,
            ):
                for i in rng(NT, "F"):
                    op = prps.tile([128, C], F32, tag="op")
                    for cc in range(2):
                        for oc in range(2):
                            nc.tensor.matmul(
                                op[:, oc * 512 : (oc + 1) * 512],
                                lhsT=oT_sb[:, cc, i * 128 : (i + 1) * 128],
                                rhs=pwT_sb[:, cc, oc * 512 : (oc + 1) * 512],
                                start=(cc == 0),
                                stop=(cc == 1),
                            )
                    ot = fwork.tile([128, C], F32, tag="ot")
                    if with_pb:
                        nc.vector.tensor_add(out=ot, in0=op, in1=pb_rep)
                    else:
                        nc.scalar.copy(out=ot, in_=op)
                    nc.sync.dma_start(out=out_p[i * 128 : (i + 1) * 128, :], in_=ot)

    nc.compile()
    return nc


def _rope_tables(n_w, n_b, with_b, reps=NH):
    inv = 1.0 / (10000.0 ** (np.arange(0, HD, 2, dtype=np.float64) / HD))
    ang = np.arange(NIN, dtype=np.float64)[:, None] * inv[None, :]  # (NIN, 32)
    cos_h = np.cos(ang)
    sin_h = np.sin(ang)
    w1, w2 = n_w[:32].astype(np.float64), n_w[32:].astype(np.float64)
    b1, b2 = n_b[:32].astype(np.float64), n_b[32:].astype(np.float64)
    kinds = [w1 * cos_h, w2 * sin_h, w2 * cos_h, w1 * sin_h]
    if with_b:
        kinds += [b1 * cos_h - b2 * sin_h, b2 * cos_h + b1 * sin_h]
    t = np.stack(kinds, axis=1)  # (NIN, k, 32)
    t = np.repeat(t[:, :, None, :], reps, axis=2)  # (NIN, k, reps, 32)
    return np.ascontiguousarray(t.astype(np.float16))


_NC_CACHE = {}


def kernel(x, qkv_w, qn_w, qn_b, kn_w, kn_b, proj_w, proj_b):
    x = np.asarray(x, np.float32)
    qkv_w = np.asarray(qkv_w, np.float32)
    proj_w = np.asarray(proj_w, np.float32)
    proj_b = np.asarray(proj_b, np.float32)
    qn_w = np.asarray(qn_w, np.float32)
    qn_b = np.asarray(qn_b, np.float32)
    kn_w = np.asarray(kn_w, np.float32)
    kn_b = np.asarray(kn_b, np.float32)

    with_qb = bool(np.any(qn_b != 0))
    with_kb = bool(np.any(kn_b != 0))
    shared_t = (
        with_qb == with_kb
        and np.array_equal(qn_w, kn_w)
        and np.array_equal(qn_b, kn_b)
    )
    with_pb = bool(np.any(proj_b != 0))
    key = (with_qb, with_kb, shared_t, with_pb)
    if key not in _NC_CACHE:
        _NC_CACHE[key] = build_nc(with_qb, with_kb, shared_t, with_pb)
    nc = _NC_CACHE[key]

    tq = _rope_tables(qn_w, qn_b, with_qb, reps=2 * NH if shared_t else NH)
    tk = None if shared_t else _rope_tables(kn_w, kn_b, with_kb)
    ident = np.eye(128, dtype=np.float16)

    in_maps = []
    for core in range(NCORES):
        b, g = core // GH, core % GH
        rows = slice(g * NH * HD, (g + 1) * NH * HD)
        w_core = np.ascontiguousarray(
            np.concatenate([qkv_w[rows], qkv_w[C:][rows], qkv_w[2 * C :][rows]], 0)
        )
        im = {
            "x32": np.ascontiguousarray(x[b]),
            "w32": w_core,
            "pw32": np.ascontiguousarray(proj_w[:, rows]),
            "pb": proj_b if g == 0 else np.zeros_like(proj_b),
            "tq": tq,
            "ident": ident,
        }
        if tk is not None:
            im["tk"] = tk
        in_maps.append(im)

    res = bass_utils.run_bass_kernel_spmd(nc, in_maps, core_ids=list(range(NCORES)))
    parts = [r["out_p"] for r in res.results]
    out = np.stack(
        [np.sum(parts[b * GH : (b + 1) * GH], axis=0, dtype=np.float32) for b in range(B)]
    )
    return out.astype(np.float32)


if __name__ == "__main__":
    rng = np.random.default_rng(0)
    ins = {
        "x": rng.standard_normal((B, N, C), np.float32),
        "qkv_w": (rng.standard_normal((3 * C, C), np.float32) / math.sqrt(C)).astype(
            np.float32
        ),
        "qn_w": np.ones(HD, np.float32),
        "qn_b": np.zeros(HD, np.float32),
        "kn_w": np.ones(HD, np.float32),
        "kn_b": np.zeros(HD, np.float32),
        "proj_w": (rng.standard_normal((C, C), np.float32) / math.sqrt(C)).astype(
            np.float32
        ),
        "proj_b": np.zeros(C, np.float32),
    }
    o = kernel(**ins)
    print(o.shape, o.dtype)


# revision 30
# speedup vs baseline: 3.1859x; 1.0115x over previous
"""Trainium2 Bass kernel for nn_Attention_66949950210549.

Dense transformer attention block:
  qkv = x @ qkv_w.T ; per-head LN on q,k ; RoPE (positions restart at N/2) ;
  softmax(q k^T * HD^-0.5 + cross-block log(0.5) bias) @ v ; proj.

Sharding: 8 cores = 2 (batch) x 4 (head groups of 4 heads).  Each core
computes its batch's qkv for its 4 heads, attention, and a partial
projection (row-parallel over the head channels); the host sums the 4
partials per batch (the proj bias is fed to exactly one core per batch).

Per-core layout strategy (fp16 attention core, fp32 accumulation):
  - all loads are SWDGE cast-on-DMA (f32 DRAM -> f16 SBUF); x / qkv_w /
    proj_w are transposed on-chip with batched PE transposes (identity
    matmul), PSUM->SBUF copies routed to the otherwise-idle ACT engine.
  - qkv matmul in natural (n, j) orientation; LN via one ACT square +
    grouped DVE tensor_reduce sums, applied on ACT as Identity with
    per-partition scale=rstd, bias=-mu*rstd; RoPE via host-precomputed
    cos/sin tables with the LN weights folded in (single shared table
    when qn and kn params match).
  - q/k re-transposed to (d, n) with head PAIRS packed into partitions
    0-63 / 64-127, so the K=64 scoresT matmuls auto-row-tile into
    concurrent PE row-groups.
  - exp on ACT straight out of PSUM with softmax scale and the
    cross-block log(0.5) bias folded into the activation (no max pass:
    LN bounds |score| <= 8, exp is overflow-safe in fp32).
  - AV: v-chunk stationary augmented with a ones column (M=65) so the
    softmax denominator accumulates in PSUM row 64 for free; normalize
    by the reciprocal rowsum (partition-broadcast via a DRAM bounce),
    pack oT (c_in, n), project with transposed proj weights, DMA out.
"""

import math
import os
import sys

sys.path.insert(0, "/opt/trn_rl_repo")

PHASES = os.environ.get("BASS_PHASES", "ABCDEF")

import numpy as np

import concourse.bacc as bacc
import concourse.bass as bass
import concourse.tile as tile
from concourse import bass_utils, mybir

B, N, C = 2, 2048, 1024
H, HD = 16, 64
NCORES = 8
GH = 4  # head-group count (cores per batch)
NH = H // GH  # heads per core = 4
J = 3 * NH * HD  # qkv rows per core = 768
NIN = N // 2  # rope positions restart here
NT = N // 128  # 16 row tiles
CCH = C // 128  # 8 contraction chunks
LOG_COND = math.log(0.5)
EPS = 1e-5
SCALE = HD ** -0.5  # 0.125

F32 = mybir.dt.float32
F16 = mybir.dt.float16
AF = mybir.ActivationFunctionType
AX = mybir.AxisListType
ALU = mybir.AluOpType


def build_nc(with_qb: bool, with_kb: bool, shared_t: bool = False, with_pb: bool = True):
    nc = bacc.Bacc("TRN2", target_bir_lowering=False, debug=False)

    x32 = nc.dram_tensor("x32", [N, C], F32, kind="ExternalInput")
    w32 = nc.dram_tensor("w32", [J, C], F32, kind="ExternalInput")
    pw32 = nc.dram_tensor("pw32", [C, NH * HD], F32, kind="ExternalInput")
    pb = nc.dram_tensor("pb", [C], F32, kind="ExternalInput")
    nkinds_q = 6 if with_qb else 4
    nkinds_k = 6 if with_kb else 4
    nrep = 2 * NH if shared_t else NH
    tq = nc.dram_tensor("tq", [NIN, nkinds_q, nrep, 32], F16, kind="ExternalInput")
    tk = None
    if not shared_t:
        tk = nc.dram_tensor("tk", [NIN, nkinds_k, NH, 32], F16, kind="ExternalInput")
    ident = nc.dram_tensor("ident", [128, 128], F16, kind="ExternalInput")
    out_p = nc.dram_tensor("out_p", [N, C], F32, kind="ExternalOutput")

    def rng(n, ph):
        return range(n if ph in PHASES else 0)

    with tile.TileContext(nc) as tc:
        with (
            tc.tile_pool(name="persist", bufs=1) as persist,
            tc.tile_pool(name="dram", bufs=1, space="DRAM") as dram,
        ):
            # ---- persistent SBUF tensors --------------------------------
            pwT_sb = persist.tile([128, 2, C], F16)  # proj_w^T (c_in, c_out)
            pb_rep = persist.tile([128, C], F32)  # bias replicated over parts
            v_sb = persist.tile([128, NT, NH, HD + 1], F16)  # v + ones col
            qT_sb = persist.tile([128, 2, N], F16)  # head-pair packed q^T
            kT_sb = persist.tile([128, 2, N], F16)
            oT_sb = persist.tile([128, 2, N], F16)  # head-pair packed o^T

            cst = persist.tile([128, 3], F32)
            nc.vector.memset(cst[:, 0:1], EPS)
            nc.vector.memset(cst[:, 1:2], 0.0)
            nc.vector.memset(cst[:, 2:3], LOG_COND)
            nc.const_aps.aps[(F32, EPS)] = cst[:, 0:1]
            nc.const_aps.aps[(F32, 0.0)] = cst[:, 1:2]
            nc.const_aps.aps[(F32, LOG_COND)] = cst[:, 2:3]

            pb_ap = pb[:]
            pb_bcast = bass.AP(
                tensor=pb_ap.tensor,
                offset=pb_ap.offset,
                ap=[[0, 128]] + list(pb_ap.ap),
            )
            nc.gpsimd.dma_start(out=pb_rep, in_=pb_bcast)
            nc.vector.memset(v_sb[:, :, :, HD : HD + 1], 1.0)
            id_sb = persist.tile([128, 128], F16)
            nc.sync.dma_start(out=id_sb, in_=ident[:, :])

            with (
                tc.tile_pool(name="wprep", bufs=4) as wprep,
                tc.tile_pool(name="mm1", bufs=1) as mm1,
            ):
                wT_sb = mm1.tile([128, CCH, J], F16)  # qkv_w^T (c, j)
                xT_sb = mm1.tile([128, CCH, N], F16)  # x^T (c, n)
                tq_sb = mm1.tile([128, NIN // 128, nkinds_q, nrep, 32], F16)
                nc.sync.dma_start(
                    out=tq_sb, in_=tq.rearrange("(t p) k h d -> p t k h d", p=128)
                )
                tk_sb = None
                if not shared_t:
                    tk_sb = mm1.tile([128, NIN // 128, nkinds_k, NH, 32], F16)
                    nc.sync.dma_start(
                        out=tk_sb, in_=tk.rearrange("(t p) k h d -> p t k h d", p=128)
                    )

                # ---- weights: cast-on-DMA load + PE transpose -----------
                with tc.tile_pool(name="tpps", bufs=3, space="PSUM") as tpps:
                    for jt in rng(J // 128, "B"):
                        wt16 = wprep.tile([128, C], F16, tag="w16t")
                        nc.gpsimd.dma_start(
                            out=wt16, in_=w32[jt * 128 : (jt + 1) * 128, :]
                        )
                        for cg in range(2):
                            tp = tpps.tile([128, 4, 128], F16, tag="tp")
                            for k in range(4):
                                ct = cg * 4 + k
                                nc.tensor.transpose(
                                    tp[:, k, :],
                                    wt16[:, ct * 128 : (ct + 1) * 128],
                                    id_sb,
                                )
                            nc.vector.tensor_copy(
                                out=wT_sb[
                                    :, cg * 4 : (cg + 1) * 4, jt * 128 : (jt + 1) * 128
                                ],
                                in_=tp,
                            )
                    for pt in rng(C // 128, "B"):
                        pwt16 = wprep.tile([128, NH * HD], F16, tag="pw16t")
                        nc.gpsimd.dma_start(
                            out=pwt16, in_=pw32[pt * 128 : (pt + 1) * 128, :]
                        )
                        tp = tpps.tile([128, 4, 128], F16, tag="tp")
                        for cc in range(2):
                            nc.tensor.transpose(
                                tp[:, cc, :],
                                pwt16[:, cc * 128 : (cc + 1) * 128],
                                id_sb,
                            )
                        nc.vector.tensor_copy(
                            out=pwT_sb[:, 0:2, pt * 128 : (pt + 1) * 128],
                            in_=tp[:, 0:2, :],
                        )

                    # ---- phase C: x load/cast + PE transpose ------------
                    for i in rng(NT, "C"):
                        x16 = wprep.tile([128, C], F16, tag="x16t")
                        nc.gpsimd.dma_start(
                            out=x16, in_=x32[i * 128 : (i + 1) * 128, :]
                        )
                        for cg in range(2):
                            tp = tpps.tile([128, 4, 128], F16, tag="tp")
                            for k in range(4):
                                ct = cg * 4 + k
                                nc.tensor.transpose(
                                    tp[:, k, :],
                                    x16[:, ct * 128 : (ct + 1) * 128],
                                    id_sb,
                                )
                            nc.vector.tensor_copy(
                                out=xT_sb[
                                    :, cg * 4 : (cg + 1) * 4, i * 128 : (i + 1) * 128
                                ],
                                in_=tp,
                            )

                # ---- phase D: qkv matmul + LN + rope ------------------------
                with (
                    tc.tile_pool(name="qkvps", bufs=3, space="PSUM") as qkvps,
                    tc.tile_pool(name="tpps2", bufs=2, space="PSUM") as tpps2,
                    tc.tile_pool(name="dwork", bufs=4) as dwork,
                ):
                    for i in rng(NT, "D"):
                        qp = qkvps.tile([128, 512], F32, tag="qp")
                        vp = qkvps.tile([128, 256], F32, tag="vp")
                        for cc in range(CCH):
                            nc.tensor.matmul(
                                qp,
                                lhsT=xT_sb[:, cc, i * 128 : (i + 1) * 128],
                                rhs=wT_sb[:, cc, 0:512],
                                start=(cc == 0),
                                stop=(cc == CCH - 1),
                            )
                            nc.tensor.matmul(
                                vp,
                                lhsT=xT_sb[:, cc, i * 128 : (i + 1) * 128],
                                rhs=wT_sb[:, cc, 512:768],
                                start=(cc == 0),
                                stop=(cc == CCH - 1),
                            )
                        # layernorm on the 8 (q,k) head groups:
                        # grouped sums on DVE, apply on ACT (scale/bias form)
                        qk_sb = dwork.tile([128, 2 * NH, HD], F16, tag="qk")
                        sq = dwork.tile([128, 2 * NH, HD], F32, tag="sq")
                        sums = dwork.tile([128, 4, 2 * NH], F32, tag="sums")
                        qp3 = qp.rearrange("p (g d) -> p g d", g=2 * NH)
                        nc.scalar.square(out=sq, in_=qp3)
                        nc.vector.tensor_reduce(
                            out=sums[:, 0, :], in_=qp3, axis=AX.X, op=ALU.add
                        )
                        nc.vector.tensor_reduce(
                            out=sums[:, 1, :], in_=sq, axis=AX.X, op=ALU.add
                        )
                        # mu = s/64 ; var = ss/64 - mu^2 ; rstd = rsqrt(var+eps)
                        nc.vector.tensor_scalar_mul(
                            out=sums[:, 0, :], in0=sums[:, 0, :], scalar1=1.0 / HD
                        )
                        nc.vector.tensor_scalar_mul(
                            out=sums[:, 1, :], in0=sums[:, 1, :], scalar1=1.0 / HD
                        )
                        nc.vector.tensor_mul(
                            out=sums[:, 2, :], in0=sums[:, 0, :], in1=sums[:, 0, :]
                        )
                        nc.vector.tensor_sub(
                            out=sums[:, 1, :], in0=sums[:, 1, :], in1=sums[:, 2, :]
                        )
                        nc.scalar.activation(
                            out=sums[:, 1, :], in_=sums[:, 1, :], func=AF.Sqrt, bias=EPS
                        )
                        nc.vector.reciprocal(out=sums[:, 1, :], in_=sums[:, 1, :])
                        # nb = -mu * rstd  (per-partition bias for the ACT apply)
                        nc.vector.tensor_mul(
                            out=sums[:, 2, :], in0=sums[:, 0, :], in1=sums[:, 1, :]
                        )
                        nc.vector.tensor_scalar_mul(
                            out=sums[:, 2, :], in0=sums[:, 2, :], scalar1=-1.0
                        )
                        for g in range(2 * NH):
                            nc.scalar.activation(
                                out=qk_sb[:, g, :],
                                in_=qp[:, g * HD : (g + 1) * HD],
                                func=AF.Identity,
                                bias=sums[:, 2, g : g + 1],
                                scale=sums[:, 1, g : g + 1],
                            )
                        # rope (tables carry the LN weights already)
                        qkr = dwork.tile([128, 2 * NH, HD], F16, tag="qkr")
                        r = i % (NIN // 128)
                        if shared_t:
                            groups = ((tq_sb, 0, 2 * NH, with_qb),)
                        else:
                            groups = (
                                (tq_sb, 0, NH, with_qb),
                                (tk_sb, NH, NH, with_kb),
                            )
                        for tsb, base, gn, wb in groups:
                            a1 = qk_sb[:, base : base + gn, 0:32]
                            a2 = qk_sb[:, base : base + gn, 32:64]
                            o1 = qkr[:, base : base + gn, 0:32]
                            o2 = qkr[:, base : base + gn, 32:64]
                            t_full = dwork.tile(
                                [128, 2 * NH, 32], F16, tag="ropetmp", name="ropetmp"
                            )
                            t = t_full[:, 0:gn, :]
                            nc.vector.tensor_mul(out=t, in0=a1, in1=tsb[:, r, 0])
                            nc.vector.tensor_mul(out=o1, in0=a2, in1=tsb[:, r, 1])
                            nc.vector.tensor_sub(out=o1, in0=t, in1=o1)
                            nc.vector.tensor_mul(out=t, in0=a2, in1=tsb[:, r, 2])
                            nc.vector.tensor_mul(out=o2, in0=a1, in1=tsb[:, r, 3])
                            nc.vector.tensor_add(out=o2, in0=t, in1=o2)
                            if wb:
                                nc.vector.tensor_add(out=o1, in0=o1, in1=tsb[:, r, 4])
                                nc.vector.tensor_add(out=o2, in0=o2, in1=tsb[:, r, 5])
                        # qT/kT via PE transpose (head pairs packed)
                        tp = tpps2.tile([128, 4, 128], F16, tag="tpqk")
                        for hp in range(2):
                            nc.tensor.transpose(
                                tp[:, hp, :],
                                qkr[:, 2 * hp : 2 * hp + 2, :],
                                id_sb,
                            )
                            nc.tensor.transpose(
                                tp[:, 2 + hp, :],
                                qkr[:, NH + 2 * hp : NH + 2 * hp + 2, :],
                                id_sb,
                            )
                        nc.scalar.copy(
                            out=qT_sb[:, 0:2, i * 128 : (i + 1) * 128],
                            in_=tp[:, 0:2, :],
                        )
                        nc.scalar.copy(
                            out=kT_sb[:, 0:2, i * 128 : (i + 1) * 128],
                            in_=tp[:, 2:4, :],
                        )
                        # v (cast to fp16, strided into the ones-augmented slots)
                        nc.vector.tensor_copy(
                            out=v_sb[:, i, :, 0:HD],
                            in_=vp.rearrange("p (h d) -> p h d", h=NH),
                        )

            # ---- phase E: attention -------------------------------------
            with (
                tc.tile_pool(name="scps", bufs=1, space="PSUM") as scps,
                tc.tile_pool(name="avps", bufs=1, space="PSUM") as avps,
                tc.tile_pool(name="epool", bufs=2) as epool,
                tc.tile_pool(name="nwork", bufs=2) as nwork,
            ):
                for nqh in rng(2, "E"):
                    for hp in range(2):
                        e_t = [
                            epool.tile([128, NT, 1024], F16, tag=f"E{z}", name=f"E{z}")
                            for z in range(2)
                        ]
                        for kc in range(NT):
                            bias = 0.0 if ((kc < 8) == (nqh == 0)) else LOG_COND
                            for z in range(2):
                                sp = scps.tile(
                                    [128, 1024], F32, tag=f"s{z}", name=f"s{z}"
                                )
                                for nqc in range(2):
                                    nq0 = nqh * 1024 + nqc * 512
                                    nc.tensor.matmul(
                                        sp[:, nqc * 512 : (nqc + 1) * 512],
                                        lhsT=kT_sb[
                                            z * 64 : (z + 1) * 64,
                                            hp,
                                            kc * 128 : (kc + 1) * 128,
                                        ],
                                        rhs=qT_sb[
                                            z * 64 : (z + 1) * 64, hp, nq0 : nq0 + 512
                                        ],
                                        start=True,
                                        stop=True,
                                    )
                                nc.scalar.activation(
                                    out=e_t[z][:, kc, :],
                                    in_=sp,
                                    func=AF.Exp,
                                    bias=bias,
                                    scale=SCALE,
                                )
                        av_t = [
                            avps.tile([128, 1024], F32, tag=f"av{z}", name=f"av{z}")
                            for z in range(2)
                        ]
                        for kc in range(NT):
                            for z in range(2):
                                for nqc in range(2):
                                    nc.tensor.matmul(
                                        av_t[z][
                                            0 : HD + 1, nqc * 512 : (nqc + 1) * 512
                                        ],
                                        lhsT=v_sb[:, kc, 2 * hp + z, :],
                                        rhs=e_t[z][
                                            :, kc, nqc * 512 : (nqc + 1) * 512
                                        ],
                                        start=(kc == 0),
                                        stop=(kc == NT - 1),
                                    )
                        # normalize: o = av[0:64] * (1/av[64]) , pack into oT_sb
                        for z in range(2):
                            rs = nwork.tile([128, 1024], F32, tag="rs")
                            nc.vector.reciprocal(
                                out=rs[HD : HD + 1, :], in_=av_t[z][HD : HD + 1, :]
                            )
                            rs_d = dram.tile([1, 1024], F32, tag="rsd", name="rs_d")
                            nc.sync.dma_start(out=rs_d, in_=rs[HD : HD + 1, :])
                            rr = nwork.tile([64, 1024], F32, tag="rr")
                            rs_ap = rs_d[:]
                            nc.gpsimd.dma_start(
                                out=rr,
                                in_=bass.AP(
                                    tensor=rs_ap.tensor,
                                    offset=rs_ap.offset,
                                    ap=[[0, 64]] + list(rs_ap.ap[1:]),
                                ),
                            )
                            o16 = nwork.tile([64, 1024], F16, tag="o16")
                            nc.vector.tensor_mul(
                                out=o16, in0=av_t[z][0:HD, :], in1=rr
                            )
                            nc.sync.dma_start(
                                out=oT_sb[
                                    z * 64 : (z + 1) * 64,
                                    hp,
                                    nqh * 1024 : (nqh + 1) * 1024,
                                ],
                                in_=o16,
                            )

            # ---- phase F: projection ------------------------------------
            with (
                tc.tile_pool(name="prps", bufs=2, space="PSUM") as prps,
                tc.tile_pool(name="fwork", bufs=3) as fwork,
            ):
                for i in rng(NT, "F"):
                    op = prps.tile([128, C], F32, tag="op")
                    for cc in range(2):
                        for oc in range(2):
                            nc.tensor.matmul(
                                op[:, oc * 512 : (oc + 1) * 512],
                                lhsT=oT_sb[:, cc, i * 128 : (i + 1) * 128],
                                rhs=pwT_sb[:, cc, oc * 512 : (oc + 1) * 512],
                                start=(cc == 0),
                                stop=(cc == 1),
                            )
                    ot = fwork.tile([128, C], F32, tag="ot")
                    if with_pb:
                        nc.vector.tensor_add(out=ot, in0=op, in1=pb_rep)
                    else:
                        nc.scalar.copy(out=ot, in_=op)
                    nc.sync.dma_start(out=out_p[i * 128 : (i + 1) * 128, :], in_=ot)

    nc.compile()
    return nc


def _rope_tables(n_w, n_b, with_b, reps=NH):
    inv = 1.0 / (10000.0 ** (np.arange(0, HD, 2, dtype=np.float64) / HD))
    ang = np.arange(NIN, dtype=np.float64)[:, None] * inv[None, :]  # (NIN, 32)
    cos_h = np.cos(ang)
    sin_h = np.sin(ang)
    w1, w2 = n_w[:32].astype(np.float64), n_w[32:].astype(np.float64)
    b1, b2 = n_b[:32].astype(np.float64), n_b[32:].astype(np.float64)
    kinds = [w1 * cos_h, w2 * sin_h, w2 * cos_h, w1 * sin_h]
    if with_b:
        kinds += [b1 * cos_h - b2 * sin_h, b2 * cos_h + b1 * sin_h]
    t = np.stack(kinds, axis=1)  # (NIN, k, 32)
    t = np.repeat(t[:, :, None, :], reps, axis=2)  # (NIN, k, reps, 32)
    return np.ascontiguousarray(t.astype(np.float16))


_NC_CACHE = {}


def kernel(x, qkv_w, qn_w, qn_b, kn_w, kn_b, proj_w, proj_b):
    x = np.asarray(x, np.float32)
    qkv_w = np.asarray(qkv_w, np.float32)
    proj_w = np.asarray(proj_w, np.float32)
    proj_b = np.asarray(proj_b, np.float32)
    qn_w = np.asarray(qn_w, np.float32)
    qn_b = np.asarray(qn_b, np.float32)
    kn_w = np.asarray(kn_w, np.float32)
    kn_b = np.asarray(kn_b, np.float32)

    with_qb = bool(np.any(qn_b != 0))
    with_kb = bool(np.any(kn_b != 0))
    shared_t = (
        with_qb == with_kb
        and np.array_equal(qn_w, kn_w)
        and np.array_equal(qn_b, kn_b)
    )
    with_pb = bool(np.any(proj_b != 0))
    key = (with_qb, with_kb, shared_t, with_pb)
    if key not in _NC_CACHE:
        _NC_CACHE[key] = build_nc(with_qb, with_kb, shared_t, with_pb)
    nc = _NC_CACHE[key]

    tq = _rope_tables(qn_w, qn_b, with_qb, reps=2 * NH if shared_t else NH)
    tk = None if shared_t else _rope_tables(kn_w, kn_b, with_kb)
    ident = np.eye(128, dtype=np.float16)

    in_maps = []
    for core in range(NCORES):
        b, g = core // GH, core % GH
        rows = slice(g * NH * HD, (g + 1) * NH * HD)
        w_core = np.ascontiguousarray(
            np.concatenate([qkv_w[rows], qkv_w[C:][rows], qkv_w[2 * C :][rows]], 0)
        )
        im = {
            "x32": np.ascontiguousarray(x[b]),
            "w32": w_core,
            "pw32": np.ascontiguousarray(proj_w[:, rows]),
            "pb": proj_b if g == 0 else np.zeros_like(proj_b),
            "tq": tq,
            "ident": ident,
        }
        if tk is not None:
            im["tk"] = tk
        in_maps.append(im)

    res = bass_utils.run_bass_kernel_spmd(nc, in_maps, core_ids=list(range(NCORES)))
    parts = [r["out_p"] for r in res.results]
    out = np.stack(
        [np.sum(parts[b * GH : (b + 1) * GH], axis=0, dtype=np.float32) for b in range(B)]
    )
    return out.astype(np.float32)


if __name__ == "__main__":
    rng = np.random.default_rng(0)
    ins = {
        "x": rng.standard_normal((B, N, C), np.float32),
        "qkv_w": (rng.standard_normal((3 * C, C), np.float32) / math.sqrt(C)).astype(
            np.float32
        ),
        "qn_w": np.ones(HD, np.float32),
        "qn_b": np.zeros(HD, np.float32),
        "kn_w": np.ones(HD, np.float32),
        "kn_b": np.zeros(HD, np.float32),
        "proj_w": (rng.standard_normal((C, C), np.float32) / math.sqrt(C)).astype(
            np.float32
        ),
        "proj_b": np.zeros(C, np.float32),
    }
    o = kernel(**ins)
    print(o.shape, o.dtype)


# revision 32
# speedup vs baseline: 3.2306x; 1.0140x over previous
"""Trainium2 Bass kernel for nn_Attention_66949950210549.

Dense transformer attention block:
  qkv = x @ qkv_w.T ; per-head LN on q,k ; RoPE (positions restart at N/2) ;
  softmax(q k^T * HD^-0.5 + cross-block log(0.5) bias) @ v ; proj.

Sharding: 8 cores = 2 (batch) x 4 (head groups of 4 heads).  Each core
computes its batch's qkv for its 4 heads, attention, and a partial
projection (row-parallel over the head channels); the host sums the 4
partials per batch (the proj bias is fed to exactly one core per batch).

Per-core layout strategy (fp16 attention core, fp32 accumulation):
  - all loads are SWDGE cast-on-DMA (f32 DRAM -> f16 SBUF); x / qkv_w /
    proj_w are transposed on-chip with batched PE transposes (identity
    matmul); prefix PSUM->SBUF copies go to DVE (idle there), the
    phase-D q/k transpose copies to ACT (idle there).
  - qkv matmul in natural (n, j) orientation; LN via one ACT square +
    grouped DVE tensor_reduce sums, applied on ACT as Identity with
    per-partition scale=rstd, bias=-mu*rstd; RoPE via host-precomputed
    cos/sin tables with the LN weights folded in (single shared table
    when qn and kn params match).
  - q/k re-transposed to (d, n) with head PAIRS packed into partitions
    0-63 / 64-127, so the K=64 scoresT matmuls auto-row-tile into
    concurrent PE row-groups.
  - exp on ACT straight out of PSUM with softmax scale and the
    cross-block log(0.5) bias folded into the activation (no max pass:
    LN bounds |score| <= 8, exp is overflow-safe in fp32).
  - AV: v-chunk stationary augmented with a ones column (M=65) so the
    softmax denominator accumulates in PSUM row 64 for free; normalize
    by the reciprocal rowsum (partition-broadcast via a DRAM bounce),
    pack oT (c_in, n), project with transposed proj weights, DMA out.
"""

import math
import os
import sys

sys.path.insert(0, "/opt/trn_rl_repo")

PHASES = os.environ.get("BASS_PHASES", "ABCDEF")

import numpy as np

import concourse.bacc as bacc
import concourse.bass as bass
import concourse.tile as tile
from concourse import bass_utils, mybir

B, N, C = 2, 2048, 1024
H, HD = 16, 64
NCORES = 8
GH = 4  # head-group count (cores per batch)
NH = H // GH  # heads per core = 4
J = 3 * NH * HD  # qkv rows per core = 768
NIN = N // 2  # rope positions restart here
NT = N // 128  # 16 row tiles
CCH = C // 128  # 8 contraction chunks
LOG_COND = math.log(0.5)
EPS = 1e-5
SCALE = HD ** -0.5  # 0.125

F32 = mybir.dt.float32
F16 = mybir.dt.float16
AF = mybir.ActivationFunctionType
AX = mybir.AxisListType
ALU = mybir.AluOpType


def build_nc(with_qb: bool, with_kb: bool, shared_t: bool = False, with_pb: bool = True):
    nc = bacc.Bacc("TRN2", target_bir_lowering=False, debug=False)

    x32 = nc.dram_tensor("x32", [N, C], F32, kind="ExternalInput")
    w32 = nc.dram_tensor("w32", [J, C], F32, kind="ExternalInput")
    pw32 = nc.dram_tensor("pw32", [C, NH * HD], F32, kind="ExternalInput")
    pb = nc.dram_tensor("pb", [C], F32, kind="ExternalInput")
    nkinds_q = 6 if with_qb else 4
    nkinds_k = 6 if with_kb else 4
    nrep = 2 * NH if shared_t else NH
    tq = nc.dram_tensor("tq", [NIN, nkinds_q, nrep, 32], F16, kind="ExternalInput")
    tk = None
    if not shared_t:
        tk = nc.dram_tensor("tk", [NIN, nkinds_k, NH, 32], F16, kind="ExternalInput")
    ident = nc.dram_tensor("ident", [128, 128], F16, kind="ExternalInput")
    out_p = nc.dram_tensor("out_p", [N, C], F32, kind="ExternalOutput")

    def rng(n, ph):
        return range(n if ph in PHASES else 0)

    with tile.TileContext(nc) as tc:
        with (
            tc.tile_pool(name="persist", bufs=1) as persist,
            tc.tile_pool(name="dram", bufs=1, space="DRAM") as dram,
        ):
            # ---- persistent SBUF tensors --------------------------------
            pwT_sb = persist.tile([128, 2, C], F16)  # proj_w^T (c_in, c_out)
            pb_rep = persist.tile([128, C], F32)  # bias replicated over parts
            v_sb = persist.tile([128, NT, NH, HD + 1], F16)  # v + ones col
            qT_sb = persist.tile([128, 2, N], F16)  # head-pair packed q^T
            kT_sb = persist.tile([128, 2, N], F16)
            oT_sb = persist.tile([128, 2, N], F16)  # head-pair packed o^T

            cst = persist.tile([128, 3], F32)
            nc.vector.memset(cst[:, 0:1], EPS)
            nc.vector.memset(cst[:, 1:2], 0.0)
            nc.vector.memset(cst[:, 2:3], LOG_COND)
            nc.const_aps.aps[(F32, EPS)] = cst[:, 0:1]
            nc.const_aps.aps[(F32, 0.0)] = cst[:, 1:2]
            nc.const_aps.aps[(F32, LOG_COND)] = cst[:, 2:3]

            pb_ap = pb[:]
            pb_bcast = bass.AP(
                tensor=pb_ap.tensor,
                offset=pb_ap.offset,
                ap=[[0, 128]] + list(pb_ap.ap),
            )
            nc.gpsimd.dma_start(out=pb_rep, in_=pb_bcast)
            nc.vector.memset(v_sb[:, :, :, HD : HD + 1], 1.0)
            id_sb = persist.tile([128, 128], F16)
            nc.sync.dma_start(out=id_sb, in_=ident[:, :])

            with (
                tc.tile_pool(name="wprep", bufs=6) as wprep,
                tc.tile_pool(name="mm1", bufs=1) as mm1,
            ):
                wT_sb = mm1.tile([128, CCH, J], F16)  # qkv_w^T (c, j)
                xT_sb = mm1.tile([128, CCH, N], F16)  # x^T (c, n)
                tq_sb = mm1.tile([128, NIN // 128, nkinds_q, nrep, 32], F16)
                nc.sync.dma_start(
                    out=tq_sb, in_=tq.rearrange("(t p) k h d -> p t k h d", p=128)
                )
                tk_sb = None
                if not shared_t:
                    tk_sb = mm1.tile([128, NIN // 128, nkinds_k, NH, 32], F16)
                    nc.sync.dma_start(
                        out=tk_sb, in_=tk.rearrange("(t p) k h d -> p t k h d", p=128)
                    )

                # ---- weights: cast-on-DMA load + PE transpose -----------
                with tc.tile_pool(name="tpps", bufs=3, space="PSUM") as tpps:
                    for jt in rng(J // 128, "B"):
                        wt16 = wprep.tile([128, C], F16, tag="w16t")
                        nc.gpsimd.dma_start(
                            out=wt16, in_=w32[jt * 128 : (jt + 1) * 128, :]
                        )
                        for cg in range(2):
                            tp = tpps.tile([128, 4, 128], F16, tag="tp")
                            for k in range(4):
                                ct = cg * 4 + k
                                nc.tensor.transpose(
                                    tp[:, k, :],
                                    wt16[:, ct * 128 : (ct + 1) * 128],
                                    id_sb,
                                )
                            nc.vector.tensor_copy(
                                out=wT_sb[
                                    :, cg * 4 : (cg + 1) * 4, jt * 128 : (jt + 1) * 128
                                ],
                                in_=tp,
                            )
                    for pt in rng(C // 128, "B"):
                        pwt16 = wprep.tile([128, NH * HD], F16, tag="pw16t")
                        nc.gpsimd.dma_start(
                            out=pwt16, in_=pw32[pt * 128 : (pt + 1) * 128, :]
                        )
                        tp = tpps.tile([128, 4, 128], F16, tag="tp")
                        for cc in range(2):
                            nc.tensor.transpose(
                                tp[:, cc, :],
                                pwt16[:, cc * 128 : (cc + 1) * 128],
                                id_sb,
                            )
                        nc.vector.tensor_copy(
                            out=pwT_sb[:, 0:2, pt * 128 : (pt + 1) * 128],
                            in_=tp[:, 0:2, :],
                        )

                    # ---- phase C: x load/cast + PE transpose ------------
                    for i in rng(NT, "C"):
                        x16 = wprep.tile([128, C], F16, tag="x16t")
                        nc.gpsimd.dma_start(
                            out=x16, in_=x32[i * 128 : (i + 1) * 128, :]
                        )
                        for cg in range(2):
                            tp = tpps.tile([128, 4, 128], F16, tag="tp")
                            for k in range(4):
                                ct = cg * 4 + k
                                nc.tensor.transpose(
                                    tp[:, k, :],
                                    x16[:, ct * 128 : (ct + 1) * 128],
                                    id_sb,
                                )
                            nc.vector.tensor_copy(
                                out=xT_sb[
                                    :, cg * 4 : (cg + 1) * 4, i * 128 : (i + 1) * 128
                                ],
                                in_=tp,
                            )

                # ---- phase D: qkv matmul + LN + rope ------------------------
                with (
                    tc.tile_pool(name="qkvps", bufs=3, space="PSUM") as qkvps,
                    tc.tile_pool(name="tpps2", bufs=2, space="PSUM") as tpps2,
                    tc.tile_pool(name="dwork", bufs=6) as dwork,
                ):
                    for i in rng(NT, "D"):
                        qp = qkvps.tile([128, 512], F32, tag="qp")
                        vp = qkvps.tile([128, 256], F32, tag="vp")
                        for cc in range(CCH):
                            nc.tensor.matmul(
                                qp,
                                lhsT=xT_sb[:, cc, i * 128 : (i + 1) * 128],
                                rhs=wT_sb[:, cc, 0:512],
                                start=(cc == 0),
                                stop=(cc == CCH - 1),
                            )
                            nc.tensor.matmul(
                                vp,
                                lhsT=xT_sb[:, cc, i * 128 : (i + 1) * 128],
                                rhs=wT_sb[:, cc, 512:768],
                                start=(cc == 0),
                                stop=(cc == CCH - 1),
                            )
                        # layernorm on the 8 (q,k) head groups:
                        # grouped sums on DVE, apply on ACT (scale/bias form)
                        qk_sb = dwork.tile([128, 2 * NH, HD], F16, tag="qk")
                        sq = dwork.tile([128, 2 * NH, HD], F32, tag="sq")
                        sums = dwork.tile([128, 4, 2 * NH], F32, tag="sums")
                        qp3 = qp.rearrange("p (g d) -> p g d", g=2 * NH)
                        nc.scalar.square(out=sq, in_=qp3)
                        nc.vector.tensor_reduce(
                            out=sums[:, 0, :], in_=qp3, axis=AX.X, op=ALU.add
                        )
                        nc.vector.tensor_reduce(
                            out=sums[:, 1, :], in_=sq, axis=AX.X, op=ALU.add
                        )
                        # mu = s/64 ; var = ss/64 - mu^2 ; rstd = rsqrt(var+eps)
                        nc.vector.tensor_scalar_mul(
                            out=sums[:, 0, :], in0=sums[:, 0, :], scalar1=1.0 / HD
                        )
                        nc.vector.tensor_scalar_mul(
                            out=sums[:, 1, :], in0=sums[:, 1, :], scalar1=1.0 / HD
                        )
                        nc.vector.tensor_mul(
                            out=sums[:, 2, :], in0=sums[:, 0, :], in1=sums[:, 0, :]
                        )
                        nc.vector.tensor_sub(
                            out=sums[:, 1, :], in0=sums[:, 1, :], in1=sums[:, 2, :]
                        )
                        nc.scalar.activation(
                            out=sums[:, 1, :], in_=sums[:, 1, :], func=AF.Sqrt, bias=EPS
                        )
                        nc.vector.reciprocal(out=sums[:, 1, :], in_=sums[:, 1, :])
                        # nb = -mu * rstd  (per-partition bias for the ACT apply)
                        nc.vector.tensor_mul(
                            out=sums[:, 2, :], in0=sums[:, 0, :], in1=sums[:, 1, :]
                        )
                        nc.vector.tensor_scalar_mul(
                            out=sums[:, 2, :], in0=sums[:, 2, :], scalar1=-1.0
                        )
                        for g in range(2 * NH):
                            nc.scalar.activation(
                                out=qk_sb[:, g, :],
                                in_=qp[:, g * HD : (g + 1) * HD],
                                func=AF.Identity,
                                bias=sums[:, 2, g : g + 1],
                                scale=sums[:, 1, g : g + 1],
                            )
                        # rope (tables carry the LN weights already)
                        qkr = dwork.tile([128, 2 * NH, HD], F16, tag="qkr")
                        r = i % (NIN // 128)
                        if shared_t:
                            groups = ((tq_sb, 0, 2 * NH, with_qb),)
                        else:
                            groups = (
                                (tq_sb, 0, NH, with_qb),
                                (tk_sb, NH, NH, with_kb),
                            )
                        for tsb, base, gn, wb in groups:
                            a1 = qk_sb[:, base : base + gn, 0:32]
                            a2 = qk_sb[:, base : base + gn, 32:64]
                            o1 = qkr[:, base : base + gn, 0:32]
                            o2 = qkr[:, base : base + gn, 32:64]
                            t_full = dwork.tile(
                                [128, 2 * NH, 32], F16, tag="ropetmp", name="ropetmp"
                            )
                            t = t_full[:, 0:gn, :]
                            nc.vector.tensor_mul(out=t, in0=a1, in1=tsb[:, r, 0])
                            nc.vector.tensor_mul(out=o1, in0=a2, in1=tsb[:, r, 1])
                            nc.vector.tensor_sub(out=o1, in0=t, in1=o1)
                            nc.vector.tensor_mul(out=t, in0=a2, in1=tsb[:, r, 2])
                            nc.vector.tensor_mul(out=o2, in0=a1, in1=tsb[:, r, 3])
                            nc.vector.tensor_add(out=o2, in0=t, in1=o2)
                            if wb:
                                nc.vector.tensor_add(out=o1, in0=o1, in1=tsb[:, r, 4])
                                nc.vector.tensor_add(out=o2, in0=o2, in1=tsb[:, r, 5])
                        # qT/kT via PE transpose (head pairs packed)
                        tp = tpps2.tile([128, 4, 128], F16, tag="tpqk")
                        for hp in range(2):
                            nc.tensor.transpose(
                                tp[:, hp, :],
                                qkr[:, 2 * hp : 2 * hp + 2, :],
                                id_sb,
                            )
                            nc.tensor.transpose(
                                tp[:, 2 + hp, :],
                                qkr[:, NH + 2 * hp : NH + 2 * hp + 2, :],
                                id_sb,
                            )
                        nc.scalar.copy(
                            out=qT_sb[:, 0:2, i * 128 : (i + 1) * 128],
                            in_=tp[:, 0:2, :],
                        )
                        nc.scalar.copy(
                            out=kT_sb[:, 0:2, i * 128 : (i + 1) * 128],
                            in_=tp[:, 2:4, :],
                        )
                        # v (cast to fp16, strided into the ones-augmented slots)
                        nc.vector.tensor_copy(
                            out=v_sb[:, i, :, 0:HD],
                            in_=vp.rearrange("p (h d) -> p h d", h=NH),
                        )

            # ---- phase E: attention -------------------------------------
            with (
                tc.tile_pool(name="scps", bufs=1, space="PSUM") as scps,
                tc.tile_pool(name="avps", bufs=1, space="PSUM") as avps,
                tc.tile_pool(name="epool", bufs=2) as epool,
                tc.tile_pool(name="nwork", bufs=3) as nwork,
            ):
                for nqh in rng(2, "E"):
                    for hp in range(2):
                        e_t = [
                            epool.tile([128, NT, 1024], F16, tag=f"E{z}", name=f"E{z}")
                            for z in range(2)
                        ]
                        for kc in range(NT):
                            bias = 0.0 if ((kc < 8) == (nqh == 0)) else LOG_COND
                            for z in range(2):
                                sp = scps.tile(
                                    [128, 1024], F32, tag=f"s{z}", name=f"s{z}"
                                )
                                for nqc in range(2):
                                    nq0 = nqh * 1024 + nqc * 512
                                    nc.tensor.matmul(
                                        sp[:, nqc * 512 : (nqc + 1) * 512],
                                        lhsT=kT_sb[
                                            z * 64 : (z + 1) * 64,
                                            hp,
                                            kc * 128 : (kc + 1) * 128,
                                        ],
                                        rhs=qT_sb[
                                            z * 64 : (z + 1) * 64, hp, nq0 : nq0 + 512
                                        ],
                                        start=True,
                                        stop=True,
                                    )
                                nc.scalar.activation(
                                    out=e_t[z][:, kc, :],
                                    in_=sp,
                                    func=AF.Exp,
                                    bias=bias,
                                    scale=SCALE,
                                )
                        av_t = [
                            avps.tile([128, 1024], F32, tag=f"av{z}", name=f"av{z}")
                            for z in range(2)
                        ]
                        for kc in range(NT):
                            for z in range(2):
                                for nqc in range(2):
                                    nc.tensor.matmul(
                                        av_t[z][
                                            0 : HD + 1, nqc * 512 : (nqc + 1) * 512
                                        ],
                                        lhsT=v_sb[:, kc, 2 * hp + z, :],
                                        rhs=e_t[z][
                                            :, kc, nqc * 512 : (nqc + 1) * 512
                                        ],
                                        start=(kc == 0),
                                        stop=(kc == NT - 1),
                                    )
                        # normalize: o = av[0:64] * (1/av[64]) , pack into oT_sb
                        for z in range(2):
                            rs = nwork.tile([128, 1024], F32, tag="rs")
                            nc.vector.reciprocal(
                                out=rs[HD : HD + 1, :], in_=av_t[z][HD : HD + 1, :]
                            )
                            rs_d = dram.tile([1, 1024], F32, tag="rsd", name="rs_d")
                            nc.sync.dma_start(out=rs_d, in_=rs[HD : HD + 1, :])
                            rr = nwork.tile([64, 1024], F32, tag="rr")
                            rs_ap = rs_d[:]
                            nc.gpsimd.dma_start(
                                out=rr,
                                in_=bass.AP(
                                    tensor=rs_ap.tensor,
                                    offset=rs_ap.offset,
                                    ap=[[0, 64]] + list(rs_ap.ap[1:]),
                                ),
                            )
                            o16 = nwork.tile([64, 1024], F16, tag="o16")
                            nc.vector.tensor_mul(
                                out=o16, in0=av_t[z][0:HD, :], in1=rr
                            )
                            nc.sync.dma_start(
                                out=oT_sb[
                                    z * 64 : (z + 1) * 64,
                                    hp,
                                    nqh * 1024 : (nqh + 1) * 1024,
                                ],
                                in_=o16,
                            )

            # ---- phase F: projection ------------------------------------
            with (
                tc.tile_pool(name="prps", bufs=2, space="PSUM") as prps,
                tc.tile_pool(name="fwork", bufs=4) as fwork,
            ):
                for i in rng(NT, "F"):
                    op = prps.tile([128, C], F32, tag="op")
                    for cc in range(2):
                        for oc in range(2):
                            nc.tensor.matmul(
                                op[:, oc * 512 : (oc + 1) * 512],
                                lhsT=oT_sb[:, cc, i * 128 : (i + 1) * 128],
                                rhs=pwT_sb[:, cc, oc * 512 : (oc + 1) * 512],
                                start=(cc == 0),
                                stop=(cc == 1),
                            )
                    ot = fwork.tile([128, C], F32, tag="ot")
                    if with_pb:
                        nc.vector.tensor_add(out=ot, in0=op, in1=pb_rep)
                    else:
                        nc.scalar.copy(out=ot, in_=op)
                    nc.sync.dma_start(out=out_p[i * 128 : (i + 1) * 128, :], in_=ot)

    nc.compile()
    return nc


def _rope_tables(n_w, n_b, with_b, reps=NH):
    inv = 1.0 / (10000.0 ** (np.arange(0, HD, 2, dtype=np.float64) / HD))
    ang = np.arange(NIN, dtype=np.float64)[:, None] * inv[None, :]  # (NIN, 32)
    cos_h = np.cos(ang)
    sin_h = np.sin(ang)
    w1, w2 = n_w[:32].astype(np.float64), n_w[32:].astype(np.float64)
    b1, b2 = n_b[:32].astype(np.float64), n_b[32:].astype(np.float64)
    kinds = [w1 * cos_h, w2 * sin_h, w2 * cos_h, w1 * sin_h]
    if with_b:
        kinds += [b1 * cos_h - b2 * sin_h, b2 * cos_h + b1 * sin_h]
    t = np.stack(kinds, axis=1)  # (NIN, k, 32)
    t = np.repeat(t[:, :, None, :], reps, axis=2)  # (NIN, k, reps, 32)
    return np.ascontiguousarray(t.astype(np.float16))


_NC_CACHE = {}


def kernel(x, qkv_w, qn_w, qn_b, kn_w, kn_b, proj_w, proj_b):
    x = np.asarray(x, np.float32)
    qkv_w = np.asarray(qkv_w, np.float32)
    proj_w = np.asarray(proj_w, np.float32)
    proj_b = np.asarray(proj_b, np.float32)
    qn_w = np.asarray(qn_w, np.float32)
    qn_b = np.asarray(qn_b, np.float32)
    kn_w = np.asarray(kn_w, np.float32)
    kn_b = np.asarray(kn_b, np.float32)

    with_qb = bool(np.any(qn_b != 0))
    with_kb = bool(np.any(kn_b != 0))
    shared_t = (
        with_qb == with_kb
        and np.array_equal(qn_w, kn_w)
        and np.array_equal(qn_b, kn_b)
    )
    with_pb = bool(np.any(proj_b != 0))
    key = (with_qb, with_kb, shared_t, with_pb)
    if key not in _NC_CACHE:
        _NC_CACHE[key] = build_nc(with_qb, with_kb, shared_t, with_pb)
    nc = _NC_CACHE[key]

    tq = _rope_tables(qn_w, qn_b, with_qb, reps=2 * NH if shared_t else NH)
    tk = None if shared_t else _rope_tables(kn_w, kn_b, with_kb)
    ident = np.eye(128, dtype=np.float16)

    in_maps = []
    for core in range(NCORES):
        b, g = core // GH, core % GH
        rows = slice(g * NH * HD, (g + 1) * NH * HD)
        w_core = np.ascontiguousarray(
            np.concatenate([qkv_w[rows], qkv_w[C:][rows], qkv_w[2 * C :][rows]], 0)
        )
        im = {
            "x32": np.ascontiguousarray(x[b]),
            "w32": w_core,
            "pw32": np.ascontiguousarray(proj_w[:, rows]),
            "pb": proj_b if g == 0 else np.zeros_like(proj_b),
            "tq": tq,
            "ident": ident,
        }
        if tk is not None:
            im["tk"] = tk
        in_maps.append(im)

    res = bass_utils.run_bass_kernel_spmd(nc, in_maps, core_ids=list(range(NCORES)))
    parts = [r["out_p"] for r in res.results]
    out = np.stack(
        [np.sum(parts[b * GH : (b + 1) * GH], axis=0, dtype=np.float32) for b in range(B)]
    )
    return out.astype(np.float32)


if __name__ == "__main__":
    rng = np.random.default_rng(0)
    ins = {
        "x": rng.standard_normal((B, N, C), np.float32),
        "qkv_w": (rng.standard_normal((3 * C, C), np.float32) / math.sqrt(C)).astype(
            np.float32
        ),
        "qn_w": np.ones(HD, np.float32),
        "qn_b": np.zeros(HD, np.float32),
        "kn_w": np.ones(HD, np.float32),
        "kn_b": np.zeros(HD, np.float32),
        "proj_w": (rng.standard_normal((C, C), np.float32) / math.sqrt(C)).astype(
            np.float32
        ),
        "proj_b": np.zeros(C, np.float32),
    }
    o = kernel(**ins)
    print(o.shape, o.dtype)
